# revision 19
# baseline (speedup 1.0000x reference)
"""DSMIL bag-of-instances kernel for one TRN2 chip (8 NeuronCores).

Strategy:
  - Shard N=50000 instances across 8 cores (6250 rows each, padded to
    6272 = 49*128).
  - Fold W2 into the downstream weights on device: with h = relu(x@W1+b1),
    f@M = h@(W2@M) + b2@M + b_M for M in {Wv,Wq,Wc}. Cuts matmul FLOPs ~3x.
  - Flash-attention-style softmax: exp with the LOCAL max, rescaled after a
    cross-core max, so the v-GEMM fuses with bag pooling and v is never
    materialized.
  - Critical-instance extraction without gathers: all-reduce(max) of the
    instance logits, then every core reduces hT against the
    (logits == gmax) mask row; non-owners contribute zeros to the
    all-reduce(add).
  - 4 tiny all-reduces: logits-max [2], top-feature [128,4], scores-max [2],
    bag+denominator [2,1025].
  - bf16 operands + fp32 PSUM accumulation; fp32r for fp32 matmuls.
"""

import math
import os
import sys

import numpy as np

for _p in ("/opt/trn_rl_repo",):
    if _p not in sys.path:
        sys.path.insert(0, _p)

import concourse.bacc as bacc
import concourse.mybir as mybir
import concourse.tile as tile
from concourse import masks
from concourse.ap import AP
from concourse.bass_utils import run_bass_kernel_spmd

F32 = mybir.dt.float32
F32R = mybir.dt.float32r
BF16 = mybir.dt.bfloat16
ALU = mybir.AluOpType
ACT = mybir.ActivationFunctionType
AX = mybir.AxisListType

NCORES = 8
N = 50000
NS = N // NCORES          # 6250 rows per core
NSP = 6272                # padded to 49 * 128
NB = NSP // 128           # 49
D = 1024
AD = 256                  # adaptor dim -> 2 k-tiles
A = 384                   # attn dim    -> 3 a-tiles
C = 2
CHUNKS = [(i * 512, 512) for i in range(12)] + [(6144, 128)]
RSQA = 1.0 / math.sqrt(float(A))
NEG = -1.0e30
RG = [list(range(NCORES))]

WNAMES = ("W1", "b1", "W2", "b2", "Wc", "bc", "Wq", "bq", "Wv", "bv",
          "ln_g", "ln_b", "conv_w", "conv_b")
KPH = int(os.environ.get("KPH", "9"))  # debug phase limit


class _EarlyOut(Exception):
    pass


def _r(ap):
    return ap.bitcast(F32R)


def _ap(t, extra, dims):
    """Custom access pattern into a pool tile (offset-aware)."""
    a = t[:]
    return AP(a.tensor, a.offset + extra, dims)


def _hap(h, extra, dims):
    """Custom access pattern into a raw DRAM handle."""
    return AP(h, extra, dims)


def _ksl(h, kd, width):
    """[128, width] row k-tile of a [1024, width] DRAM weight."""
    return h.ap()[kd * 128:(kd + 1) * 128, :]


def build():
    nc = bacc.Bacc("TRN2", target_bir_lowering=False, debug=False,
                   num_devices=NCORES)

    x_h = nc.dram_tensor("x", [NSP, D], F32, kind="ExternalInput")
    shapes = {"W1": [D, AD], "b1": [AD], "W2": [AD, D], "b2": [D],
              "Wc": [D, C], "bc": [C], "Wq": [D, A], "bq": [A],
              "Wv": [D, D], "bv": [D], "ln_g": [D], "ln_b": [D],
              "conv_w": [C, C, D], "conv_b": [C]}
    w_h = {k: nc.dram_tensor(k, shapes[k], F32, kind="ExternalInput")
           for k in WNAMES}
    score_h = nc.dram_tensor("score", [NS], F32, kind="ExternalOutput")
    logits_h = nc.dram_tensor("logits", [C], F32, kind="ExternalOutput")

    with tile.TileContext(nc) as tc:
        _body(nc, tc, x_h, w_h, score_h, logits_h)
    nc.compile()
    return nc


def _copy(eng, nc, out, in_):
    if eng == "s":
        nc.scalar.copy(out, in_)
    else:
        nc.vector.tensor_copy(out, in_)


def _body(nc, tc, x_h, w_h, score_h, logits_h):
    import contextlib
    es = contextlib.ExitStack()
    P = es.enter_context(tc.tile_pool(name="persist", bufs=1))
    DP = es.enter_context(tc.tile_pool(name="dram", bufs=1, space="DRAM"))

    # ---------------- persistent SBUF ----------------
    w1b = P.tile([128, 8 * AD], BF16, tag="w1b")
    wvp = P.tile([128, 2 * D], BF16, tag="wvp")
    wqp = P.tile([128, 2 * A], BF16, tag="wqp")
    wcp = P.tile([128, 2 * C], BF16, tag="wcp")
    b1T = P.tile([128, 2], F32, tag="b1T")
    bqT = P.tile([128, 3], F32, tag="bqT")
    bcT = P.tile([C, 1], F32, tag="bcT")
    bvp_row = P.tile([1, D], F32, tag="bvp_row")
    b2T = P.tile([128, 8], F32, tag="b2T")
    hT = P.tile([128, 2 * NSP], BF16, tag="hT")
    qT = P.tile([128, 3 * NSP], BF16, tag="qT")
    logitsT = P.tile([C, NSP], F32, tag="logitsT")
    scoresT = P.tile([C, NSP], F32, tag="scoresT")
    snat = P.tile([128, C * NB], F32, tag="snat")
    wBb = P.tile([128, C * NB], BF16, tag="wBb")
    wsum_p = P.tile([128, C], F32, tag="wsum_p")
    tophT = P.tile([128, 2 * C], F32, tag="tophT")
    tophTb = P.tile([128, 2 * C], BF16, tag="tophTb")
    topqT = P.tile([128, 3 * C], BF16, tag="topqT")
    gmaxl = P.tile([C, 1], F32, tag="gmaxl")
    lmaxs = P.tile([C, 1], F32, tag="lmaxs")
    ib16 = P.tile([128, 128], BF16, tag="ib16")
    if32 = P.tile([128, 128], F32, tag="if32")
    ones = P.tile([128, 1], F32, tag="ones")
    padw = P.tile([128, C * NB], F32, tag="padw")

    # ---------------- DRAM bounce tiles ----------------
    d_l_in = DP.tile([C], F32, tag="d_l_in")
    d_l_out = DP.tile([C], F32, tag="d_l_out")
    d_th_in = DP.tile([128, 2 * C], F32, tag="d_th_in")
    d_th_out = DP.tile([128, 2 * C], F32, tag="d_th_out")
    d_sm_in = DP.tile([C], F32, tag="d_sm_in")
    d_sm_out = DP.tile([C], F32, tag="d_sm_out")
    d_bag_in = DP.tile([C, D + 1], F32, tag="d_bag_in")
    d_bag_out = DP.tile([C, D + 1], F32, tag="d_bag_out")
    d_mask = DP.tile([C, NSP], F32, tag="d_mask")
    d_tiny = DP.tile([16], F32, tag="d_tiny")
    d_zero = DP.tile([64], F32, tag="d_zero")

    masks.make_identity(nc, ib16[:])
    masks.make_identity(nc, if32[:])
    nc.vector.memset(ones[:], 1.0)
    # pad-row mask: 1.0 everywhere, 0.0 on the 22 padded instances of the
    # last 128-block (partition range not writable by compute engines).
    nc.vector.memset(padw[:], 1.0)
    zrow = P.tile([1, 64], F32, tag="zrow")
    nc.vector.memset(zrow[:], 0.0)
    nc.gpsimd.dma_start(d_zero[:], zrow[:])
    nc.gpsimd.dma_start(
        _ap(padw, 106 * (C * NB) + (NB - 1) * C, [[C * NB, 22], [1, C]]),
        _ap(d_zero, 0, [[0, 22], [1, C]]))

    # ================= prologue: biases =================
    nc.sync.dma_start(b1T[:], _hap(w_h["b1"], 0, [[1, 128], [128, 2]]))
    bc_row = P.tile([1, C], F32, tag="bc_row")
    bq_row = P.tile([1, A], F32, tag="bq_row")
    bv_row = P.tile([1, D], F32, tag="bv_row")
    nc.sync.dma_start(bc_row[:], _hap(w_h["bc"], 0, [[0, 1], [1, C]]))
    nc.sync.dma_start(bq_row[:], _hap(w_h["bq"], 0, [[0, 1], [1, A]]))
    nc.sync.dma_start(bv_row[:], _hap(w_h["bv"], 0, [[0, 1], [1, D]]))
    nc.sync.dma_start(b2T[:], _hap(w_h["b2"], 0, [[1, 128], [128, 8]]))

    # ================= prologue: weight fold =================
    with tc.tile_pool(name="fold_keep", bufs=1) as FK:
        w2T = FK.tile([128, 8 * AD], BF16, tag="w2T")
        wvb = FK.tile([128, 8 * D], BF16, tag="wvb")
        wqb = FK.tile([128, 8 * A], BF16, tag="wqb")
        wcb = FK.tile([128, 8 * C], BF16, tag="wcb")
        b2Tb = FK.tile([128, 8], BF16, tag="b2Tb")
        nc.vector.tensor_copy(b2Tb[:], b2T[:])
        with tc.tile_pool(name="fold_sb", bufs=2) as FS:
            for kd in range(8):
                t = FS.tile([128, AD], F32, tag="w1in")
                nc.sync.dma_start(t[:], _ksl(w_h["W1"], kd, AD))
                nc.vector.tensor_copy(w1b[:, kd * AD:(kd + 1) * AD], t[:])
            with tc.tile_pool(name="fold_tp", bufs=4, space="PSUM") as FTP:
                for t2 in range(2):
                    w2in = FS.tile([128, D], F32, tag="w2in")
                    nc.sync.dma_start(
                        w2in[:], w_h["W2"].ap()[t2 * 128:(t2 + 1) * 128, :])
                    w2b = FS.tile([128, D], BF16, tag="w2b")
                    nc.vector.tensor_copy(w2b[:], w2in[:])
                    for kd in range(8):
                        pt = FTP.tile([128, 128], BF16, tag="tp")
                        nc.tensor.transpose(
                            pt[:], w2b[:, kd * 128:(kd + 1) * 128], ib16[:])
                        _copy("s" if kd % 2 == 0 else "v", nc,
                              w2T[:, kd * AD + t2 * 128: kd * AD + t2 * 128 + 128],
                              pt[:])
            for kd in range(8):
                wv_k = FS.tile([128, D], F32, tag="wv_k")
                wq_k = FS.tile([128, A], F32, tag="wq_k")
                wc_k = FS.tile([128, C], F32, tag="wc_k")
                nc.sync.dma_start(wv_k[:], _ksl(w_h["Wv"], kd, D))
                nc.sync.dma_start(wq_k[:], _ksl(w_h["Wq"], kd, A))
                nc.sync.dma_start(wc_k[:], _ksl(w_h["Wc"], kd, C))
                nc.vector.tensor_copy(wvb[:, kd * D:(kd + 1) * D], wv_k[:])
                nc.vector.tensor_copy(wqb[:, kd * A:(kd + 1) * A], wq_k[:])
                nc.vector.tensor_copy(wcb[:, kd * C:(kd + 1) * C], wc_k[:])
            with tc.tile_pool(name="fold_mm", bufs=1, space="PSUM") as FP:
                psv = [[FP.tile([128, 512], F32, tag=f"psv{m}{n2}", name=f"psv{m}{n2}")
                        for n2 in range(2)] for m in range(2)]
                psq = [FP.tile([128, A], F32, tag=f"psq{m}", name=f"psq{m}") for m in range(2)]
                psc = [FP.tile([128, C], F32, tag=f"psc{m}", name=f"psc{m}") for m in range(2)]
                for kd in range(8):
                    st, sp = kd == 0, kd == 7
                    for m in range(2):
                        lh = w2T[:, kd * AD + m * 128: kd * AD + m * 128 + 128]
                        for n2 in range(2):
                            nc.tensor.matmul(
                                psv[m][n2][:], lh,
                                wvb[:, kd * D + n2 * 512: kd * D + (n2 + 1) * 512],
                                start=st, stop=sp)
                        nc.tensor.matmul(psq[m][:], lh,
                                         wqb[:, kd * A:(kd + 1) * A],
                                         start=st, stop=sp)
                        nc.tensor.matmul(psc[m][:], lh,
                                         wcb[:, kd * C:(kd + 1) * C],
                                         start=st, stop=sp)
                for m in range(2):
                    for n2 in range(2):
                        nc.scalar.copy(
                            wvp[:, m * D + n2 * 512: m * D + (n2 + 1) * 512],
                            psv[m][n2][:])
                    nc.scalar.copy(wqp[:, m * A:(m + 1) * A], psq[m][:])
                    nc.scalar.copy(wcp[:, m * C:(m + 1) * C], psc[m][:])
            with tc.tile_pool(name="fold_b", bufs=1, space="PSUM") as FB:
                pbv = [FB.tile([1, 512], F32, tag=f"pbv{n2}", name=f"pbv{n2}") for n2 in range(2)]
                pbq = FB.tile([1, A], F32, tag="pbq")
                pbc = FB.tile([1, C], F32, tag="pbc")
                for kd in range(8):
                    st, sp = kd == 0, kd == 7
                    lh = b2Tb[:, kd:kd + 1]
                    for n2 in range(2):
                        nc.tensor.matmul(
                            pbv[n2][:], lh,
                            wvb[:, kd * D + n2 * 512: kd * D + (n2 + 1) * 512],
                            start=st, stop=sp)
                    nc.tensor.matmul(pbq[:], lh, wqb[:, kd * A:(kd + 1) * A],
                                     start=st, stop=sp)
                    nc.tensor.matmul(pbc[:], lh, wcb[:, kd * C:(kd + 1) * C],
                                     start=st, stop=sp)
                for n2 in range(2):
                    nc.vector.tensor_add(
                        bvp_row[:, n2 * 512:(n2 + 1) * 512], pbv[n2][:],
                        bv_row[:, n2 * 512:(n2 + 1) * 512])
                bqp_row = FK.tile([1, A], F32, tag="bqp_row")
                bcp_row = FK.tile([1, C], F32, tag="bcp_row")
                nc.vector.tensor_add(bqp_row[:], pbq[:], bq_row[:])
                nc.vector.tensor_add(bcp_row[:], pbc[:], bc_row[:])
                # bq' -> column layout [128, 3]; bc' -> [2, 1] via DRAM bounce
                nc.gpsimd.dma_start(_ap(d_tiny, 0, [[0, 1], [1, C]]), bcp_row[:])
                nc.gpsimd.dma_start(bcT[:], _ap(d_tiny, 0, [[1, C], [1, 1]]))
                nc.gpsimd.dma_start(_ap(d_mask, 0, [[0, 1], [1, A]]), bqp_row[:])
                nc.gpsimd.dma_start(bqT[:], _ap(d_mask, 0, [[1, 128], [128, 3]]))

    # ================= pass A: hT, qT, logitsT =================
    esA = contextlib.ExitStack()
    XP = esA.enter_context(tc.tile_pool(name="xa", bufs=4))
    XB = esA.enter_context(tc.tile_pool(name="xbf", bufs=8))
    XT = esA.enter_context(tc.tile_pool(name="xT", bufs=2))
    TP = esA.enter_context(tc.tile_pool(name="tpA", bufs=2, space="PSUM"))
    MA = esA.enter_context(tc.tile_pool(name="mmA", bufs=2, space="PSUM"))
    SA = esA.enter_context(tc.tile_pool(name="smA", bufs=1, space="PSUM"))

    def stage2(xTt, off, nx):
        for m in range(2):
            ph = MA.tile([128, 512], F32, tag="ph")
            for kd in range(8):
                nc.tensor.matmul(
                    ph[:, :nx],
                    w1b[:, kd * AD + m * 128: kd * AD + m * 128 + 128],
                    xTt[:, kd * nx: kd * nx + nx],
                    start=(kd == 0), stop=(kd == 7))
            nc.scalar.activation(hT[:, m * NSP + off: m * NSP + off + nx],
                                 ph[:, :nx], ACT.Relu,
                                 bias=b1T[:, m:m + 1], scale=1.0)
        for a in range(3):
            pq = MA.tile([128, 512], F32, tag="pq")
            for k in range(2):
                nc.tensor.matmul(
                    pq[:, :nx],
                    wqp[:, k * A + a * 128: k * A + a * 128 + 128],
                    hT[:, k * NSP + off: k * NSP + off + nx],
                    start=(k == 0), stop=(k == 1))
            nc.scalar.activation(qT[:, a * NSP + off: a * NSP + off + nx],
                                 pq[:, :nx], ACT.Identity,
                                 bias=bqT[:, a:a + 1], scale=1.0)
        pl = SA.tile([C, 512], F32, tag="pl")
        for k in range(2):
            nc.tensor.matmul(pl[:, :nx], wcp[:, k * C:(k + 1) * C],
                             hT[:, k * NSP + off: k * NSP + off + nx],
                             start=(k == 0), stop=(k == 1))
        nc.scalar.activation(logitsT[:, off:off + nx], pl[:, :nx],
                             ACT.Identity, bias=bcT[:], scale=1.0)

    prev = None
    for (off, nx) in CHUNKS:
        nblk = nx // 128
        xbs = []
        for b in range(nblk):
            xt = XP.tile([128, D], F32, tag="x")
            nc.sync.dma_start(
                xt[:], x_h.ap()[off + b * 128: off + (b + 1) * 128, :])
            xb = XB.tile([128, D], BF16, tag="xb")
            nc.vector.tensor_copy(xb[:], xt[:])
            xbs.append(xb)
        xTt = XT.tile([128, 8 * 512], BF16, tag="xT")
        nci = 0
        for kd in range(8):
            for b in range(nblk):
                pt = TP.tile([128, 128], BF16, tag="tp")
                nc.tensor.transpose(
                    pt[:], xbs[b][:, kd * 128:(kd + 1) * 128], ib16[:])
                _copy("s" if nci % 2 == 0 else "v", nc,
                      xTt[:, kd * nx + b * 128: kd * nx + (b + 1) * 128], pt[:])
                nci += 1
        if prev is not None:
            stage2(*prev)
        prev = (xTt, off, nx)
    stage2(*prev)
    esA.close()

    def _dummy_out():
        nc.sync.dma_start(score_h.ap()[:], logitsT[0:1, 0:NS])
        nc.sync.dma_start(logits_h.ap()[:], logitsT[0:2, 0:1])

    if KPH < 2:
        _dummy_out()
        es.close()
        return

    # ================= critical instance =================
    try:
      with tc.tile_pool(name="ext_sb", bufs=1) as ES, \
            tc.tile_pool(name="ext_ps", bufs=2, space="PSUM") as EP:
        nc.vector.memset(logitsT[:, NS:NSP], NEG)
        lmaxl = ES.tile([C, 1], F32, tag="lmaxl")
        nc.vector.reduce_max(lmaxl[:], logitsT[:], axis=AX.X)
        nc.gpsimd.dma_start(d_l_in[:], lmaxl[:])
        nc.gpsimd.collective_compute(
            "AllReduce", ALU.max, replica_groups=RG,
            ins=[d_l_in[:]], outs=[d_l_out[:]])
        nc.gpsimd.dma_start(gmaxl[:], _ap(d_l_out, 0, [[1, C], [1, 1]]))
        if KPH == 11:
            raise _EarlyOut()
        maskT = ES.tile([C, NSP], F32, tag="maskT")
        nc.vector.tensor_scalar(maskT[:], logitsT[:], gmaxl[:], None,
                                op0=ALU.is_equal)
        nc.sync.dma_start(d_mask[:], maskT[:])
        junk = ES.tile([128, NSP], BF16, tag="junk")
        for c in range(C):
            mb = ES.tile([128, NSP], F32, tag="mb")
            nc.sync.dma_start(mb[:], _ap(d_mask, c * NSP, [[0, 128], [1, NSP]]))
            for k in range(2):
                nc.vector.tensor_mul(junk[:], hT[:, k * NSP:(k + 1) * NSP], mb[:])
                nc.vector.tensor_reduce(
                    tophT[:, k * C + c: k * C + c + 1], junk[:],
                    axis=AX.X, op=ALU.add)
        if KPH == 12:
            raise _EarlyOut()
        nc.sync.dma_start(d_th_in[:], tophT[:])
        nc.gpsimd.collective_compute(
            "AllReduce", ALU.add, replica_groups=RG,
            ins=[d_th_in[:]], outs=[d_th_out[:]])
        nc.sync.dma_start(tophT[:], d_th_out[:])
        nc.vector.tensor_copy(tophTb[:], tophT[:])
        for a in range(3):
            pt = EP.tile([128, C], F32, tag="ptq")
            for k in range(2):
                nc.tensor.matmul(pt[:],
                                 wqp[:, k * A + a * 128: k * A + a * 128 + 128],
                                 tophTb[:, k * C:(k + 1) * C],
                                 start=(k == 0), stop=(k == 1))
            nc.scalar.activation(topqT[:, a * C:(a + 1) * C], pt[:],
                                 ACT.Identity, bias=bqT[:, a:a + 1], scale=1.0)
    except _EarlyOut:
        _dummy_out()
        es.close()
        return

    if KPH < 3:
        _dummy_out()
        es.close()
        return

    # ================= pass B1: scores =================
    with tc.tile_pool(name="b1_ps", bufs=2, space="PSUM") as BP, \
            tc.tile_pool(name="b1_ps2", bufs=4, space="PSUM") as BP2, \
            tc.tile_pool(name="b1_sb", bufs=1) as BS:
        for (off, nx) in CHUNKS:
            ps = BP.tile([C, 512], F32, tag="ps")
            for a in range(3):
                nc.tensor.matmul(ps[:, :nx], topqT[:, a * C:(a + 1) * C],
                                 qT[:, a * NSP + off: a * NSP + off + nx],
                                 start=(a == 0), stop=(a == 2))
            nc.scalar.copy(scoresT[:, off:off + nx], ps[:, :nx])
            for b in range(nx // 128):
                blk = off // 128 + b
                p2 = BP2.tile([128, C], F32, tag="p2")
                for a in range(3):
                    nc.tensor.matmul(
                        p2[:],
                        qT[:, a * NSP + blk * 128: a * NSP + (blk + 1) * 128],
                        topqT[:, a * C:(a + 1) * C],
                        start=(a == 0), stop=(a == 2))
                nc.scalar.copy(snat[:, blk * C:(blk + 1) * C], p2[:])
        nc.vector.memset(scoresT[:, NS:NSP], NEG)
        nc.sync.dma_start(score_h.ap()[:], scoresT[0:1, 0:NS])
        nc.vector.reduce_max(lmaxs[:], scoresT[:], axis=AX.X)
        # wB = exp((snat - local_max) / sqrt(A))
        nc.gpsimd.dma_start(d_sm_in[:], lmaxs[:])
        lrep = BS.tile([128, C * NB], F32, tag="lrep")
        nc.sync.dma_start(lrep[:], _ap(d_sm_in, 0, [[0, 128], [0, NB], [1, C]]))
        sd = BS.tile([128, C * NB], F32, tag="sd")
        nc.vector.tensor_tensor(sd[:], snat[:], lrep[:], op=ALU.subtract)
        wE = BS.tile([128, C * NB], F32, tag="wE")
        nc.scalar.activation(wE[:], sd[:], ACT.Exp, bias=0.0, scale=RSQA)
        nc.vector.tensor_mul(wBb[:], wE[:], padw[:])
        nc.vector.tensor_reduce(
            wsum_p[:], wBb[:].rearrange("p (b c) -> p c b", c=C),
            axis=AX.X, op=ALU.add)

    if KPH < 4:
        nc.sync.dma_start(logits_h.ap()[:], lmaxs[0:2, 0:1])
        es.close()
        return

    # ================= pass B2: v-GEMM fused with bag pooling ============
    with tc.tile_pool(name="b2_bag", bufs=1, space="PSUM") as BGP:
        pbag = [BGP.tile([C, 512], F32, tag=f"pbag{n2}", name=f"pbag{n2}") for n2 in range(2)]
        with tc.tile_pool(name="b2_ps", bufs=4, space="PSUM") as VP, \
                tc.tile_pool(name="b2_sb", bufs=4) as VS:
            pend = None
            for blk in range(NB):
                cur = []
                for n2 in range(2):
                    pv = VP.tile([128, 512], F32, tag="pv")
                    for k in range(2):
                        nc.tensor.matmul(
                            pv[:],
                            hT[:, k * NSP + blk * 128: k * NSP + (blk + 1) * 128],
                            wvp[:, k * D + n2 * 512: k * D + (n2 + 1) * 512],
                            start=(k == 0), stop=(k == 1))
                    vsb = VS.tile([128, 512], BF16, tag="vsb")
                    nc.scalar.copy(vsb[:], pv[:])
                    cur.append(vsb)
                if pend is not None:
                    for n2 in range(2):
                        nc.tensor.matmul(
                            pbag[n2][:],
                            wBb[:, pend[1] * C:(pend[1] + 1) * C],
                            pend[0][n2][:],
                            start=(pend[1] == 0), stop=(pend[1] == NB - 1))
                pend = (cur, blk)
            for n2 in range(2):
                nc.tensor.matmul(pbag[n2][:],
                                 wBb[:, pend[1] * C:(pend[1] + 1) * C],
                                 pend[0][n2][:],
                                 start=(pend[1] == 0), stop=(pend[1] == NB - 1))

        # ================= epilogue =================
        with tc.tile_pool(name="ep_sb", bufs=1) as S, \
                tc.tile_pool(name="ep_ps", bufs=1, space="PSUM") as EPP:
            nc.gpsimd.collective_compute(
                "AllReduce", ALU.max, replica_groups=RG,
                ins=[d_sm_in[:]], outs=[d_sm_out[:]])
            gmaxs = S.tile([C, 1], F32, tag="gmaxs")
            nc.gpsimd.dma_start(gmaxs[:], _ap(d_sm_out, 0, [[1, C], [1, 1]]))
            gam = S.tile([C, 1], F32, tag="gam")
            nc.vector.tensor_scalar(gam[:], lmaxs[:], gmaxs[:], None,
                                    op0=ALU.subtract)
            nc.scalar.activation(gam[:], gam[:], ACT.Exp, bias=0.0, scale=RSQA)
            pws = EPP.tile([1, C], F32, tag="pws")
            nc.tensor.matmul(pws[:], ones[:], wsum_p[:],
                             start=True, stop=True)
            ws_row = S.tile([1, C], F32, tag="ws_row")
            nc.scalar.copy(ws_row[:], pws[:])
            nc.gpsimd.dma_start(_ap(d_tiny, 4, [[0, 1], [1, C]]), ws_row[:])
            ws_col = S.tile([C, 1], F32, tag="ws_col")
            nc.gpsimd.dma_start(ws_col[:], _ap(d_tiny, 4, [[1, C], [1, 1]]))
            pack = S.tile([C, D + 1], F32, tag="pack")
            for n2 in range(2):
                nc.scalar.activation(pack[:, n2 * 512:(n2 + 1) * 512],
                                     pbag[n2][:], ACT.Copy, bias=0.0,
                                     scale=gam[:])
            nc.vector.tensor_mul(pack[:, D:D + 1], ws_col[:], gam[:])
            nc.sync.dma_start(d_bag_in[:], pack[:])
            nc.gpsimd.collective_compute(
                "AllReduce", ALU.add, replica_groups=RG,
                ins=[d_bag_in[:]], outs=[d_bag_out[:]])
            gbag = S.tile([C, D + 1], F32, tag="gbag")
            nc.sync.dma_start(gbag[:], d_bag_out[:])

            winv = S.tile([C, 1], F32, tag="winv")
            nc.vector.reciprocal(winv[:], gbag[:, D:D + 1])
            bagf = S.tile([C, D], F32, tag="bagf")
            nc.scalar.activation(bagf[:], gbag[:, 0:D], ACT.Copy, bias=0.0,
                                 scale=winv[:])
            bv2 = S.tile([C, D], F32, tag="bv2")
            nc.gpsimd.dma_start(_ap(d_mask, 0, [[0, 1], [1, D]]), bvp_row[:])
            nc.sync.dma_start(bv2[:], _ap(d_mask, 0, [[0, C], [1, D]]))
            nc.vector.tensor_add(bagf[:], bagf[:], bv2[:])
            # layernorm
            mu = S.tile([C, 1], F32, tag="mu")
            nc.vector.reduce_sum(mu[:], bagf[:], axis=AX.X)
            nc.vector.tensor_scalar_mul(mu[:], mu[:], 1.0 / D)
            xc = S.tile([C, D], F32, tag="xc")
            nc.vector.tensor_scalar(xc[:], bagf[:], mu[:], None,
                                    op0=ALU.subtract)
            sq = S.tile([C, D], F32, tag="sq")
            var = S.tile([C, 1], F32, tag="var")
            nc.vector.tensor_mul(sq[:], xc[:], xc[:])
            nc.vector.tensor_reduce(var[:], sq[:], axis=AX.X, op=ALU.add)
            nc.vector.tensor_scalar_mul(var[:], var[:], 1.0 / D)
            sdv = S.tile([C, 1], F32, tag="sdv")
            eps = S.tile([C, 1], F32, tag="eps")
            nc.vector.memset(eps[:], 1e-5)
            nc.scalar.activation(sdv[:], var[:], ACT.Sqrt, bias=eps[:], scale=1.0)
            rinv = S.tile([C, 1], F32, tag="rinv")
            nc.vector.reciprocal(rinv[:], sdv[:])
            xn = S.tile([C, D], F32, tag="xn")
            nc.scalar.activation(xn[:], xc[:], ACT.Copy, bias=0.0, scale=rinv[:])
            g2 = S.tile([C, D], F32, tag="g2")
            b2r = S.tile([C, D], F32, tag="b2r")
            nc.sync.dma_start(g2[:], _hap(w_h["ln_g"], 0, [[0, C], [1, D]]))
            nc.sync.dma_start(b2r[:], _hap(w_h["ln_b"], 0, [[0, C], [1, D]]))
            nc.vector.tensor_mul(xn[:], xn[:], g2[:])
            nc.vector.tensor_add(xn[:], xn[:], b2r[:])
            # conv contraction: blog[o] = sum_cd xn[c,d]*conv_w[o,c,d]
            cw = S.tile([C, 2 * D], F32, tag="cw")
            for o in range(C):
                nc.sync.dma_start(cw[:, o * D:(o + 1) * D],
                                  w_h["conv_w"].ap()[o, :, :])
            cv = S.tile([C, C], F32, tag="cv")
            sq2 = S.tile([C, D], F32, tag="sq2")
            for o in range(C):
                nc.vector.tensor_mul(sq2[:], xn[:], cw[:, o * D:(o + 1) * D])
                nc.vector.tensor_reduce(cv[:, o:o + 1], sq2[:],
                                        axis=AX.X, op=ALU.add)
            pcv = EPP.tile([1, C], F32, tag="pcv")
            nc.tensor.matmul(pcv[:], ones[0:C, 0:1], cv[:],
                             start=True, stop=True)
            blog_row = S.tile([1, C], F32, tag="blog_row")
            nc.scalar.copy(blog_row[:], pcv[:])
            nc.gpsimd.dma_start(_ap(d_tiny, 8, [[0, 1], [1, C]]), blog_row[:])
            blog = S.tile([C, 1], F32, tag="blog")
            nc.gpsimd.dma_start(blog[:], _ap(d_tiny, 8, [[1, C], [1, 1]]))
            cb = S.tile([C, 1], F32, tag="cb")
            nc.gpsimd.dma_start(cb[:], _hap(w_h["conv_b"], 0, [[1, C], [1, 1]]))
            nc.vector.tensor_add(blog[:], blog[:], cb[:])
            nc.vector.tensor_add(blog[:], blog[:], gmaxl[:])
            nc.scalar.activation(blog[:], blog[:], ACT.Copy, bias=0.0, scale=0.5)
            nc.gpsimd.dma_start(logits_h.ap()[:], blog[:])

    es.close()


_NC = None


def _get_nc():
    global _NC
    if _NC is None:
        _NC = build()
    return _NC


def _make_in_maps(inputs):
    x = np.asarray(inputs["x"], np.float32)
    w = {k: np.ascontiguousarray(np.asarray(inputs[k], np.float32))
         for k in WNAMES}
    in_maps = []
    for i in range(NCORES):
        xs = np.zeros((NSP, D), np.float32)
        xs[:NS] = x[i * NS:(i + 1) * NS]
        m = {"x": xs}
        m.update(w)
        in_maps.append(m)
    return in_maps


def _assemble(results):
    score = np.concatenate([results[i]["score"] for i in range(NCORES)])
    logits = np.asarray(results[0]["logits"], np.float32)
    return logits, score.astype(np.float32)


def kernel(**inputs):
    nc = _get_nc()
    res = run_bass_kernel_spmd(nc, _make_in_maps(inputs),
                               core_ids=list(range(NCORES)))
    return _assemble(res.results)


def run_traced(**inputs):
    nc = _get_nc()
    res = run_bass_kernel_spmd(nc, _make_in_maps(inputs),
                               core_ids=list(range(NCORES)), trace=True)
    return _assemble(res.results), res


# revision 20
# speedup vs baseline: 337.2840x; 337.2840x over previous
"""DSMIL bag-of-instances kernel for one TRN2 chip (8 NeuronCores).

Strategy:
  - Shard N=50000 instances across 8 cores (6250 rows each, padded to
    6272 = 49*128).
  - Fold W2 into the downstream weights on device: with h = relu(x@W1+b1),
    f@M = h@(W2@M) + b2@M + b_M for M in {Wv,Wq,Wc}. Cuts matmul FLOPs ~3x.
  - Flash-attention-style softmax: exp with the LOCAL max, rescaled after a
    cross-core max, so the v-GEMM fuses with bag pooling and v is never
    materialized.
  - Critical-instance extraction without gathers: all-reduce(max) of the
    instance logits, then every core reduces hT against the
    (logits == gmax) mask row; non-owners contribute zeros to the
    all-reduce(add).
  - 4 tiny all-reduces: logits-max [2], top-feature [128,4], scores-max [2],
    bag+denominator [2,1025].
  - bf16 operands + fp32 PSUM accumulation; fp32r for fp32 matmuls.
"""

import math
import os
import sys

import numpy as np

for _p in ("/opt/trn_rl_repo",):
    if _p not in sys.path:
        sys.path.insert(0, _p)

import concourse.bacc as bacc
import concourse.mybir as mybir
import concourse.tile as tile
from concourse import masks
from concourse.ap import AP
from concourse.bass_utils import run_bass_kernel_spmd

F32 = mybir.dt.float32
F32R = mybir.dt.float32r
BF16 = mybir.dt.bfloat16
ALU = mybir.AluOpType
ACT = mybir.ActivationFunctionType
AX = mybir.AxisListType

NCORES = 8
N = 50000
NS = N // NCORES          # 6250 rows per core
NSP = 6272                # padded to 49 * 128
NB = NSP // 128           # 49
D = 1024
AD = 256                  # adaptor dim -> 2 k-tiles
A = 384                   # attn dim    -> 3 a-tiles
C = 2
CHUNKS = [(i * 512, 512) for i in range(12)] + [(6144, 128)]
RSQA = 1.0 / math.sqrt(float(A))
NEG = -1.0e30
RG = [list(range(NCORES))]

WNAMES = ("W1", "b1", "W2", "b2", "Wc", "bc", "Wq", "bq", "Wv", "bv",
          "ln_g", "ln_b", "conv_w", "conv_b")
KPH = int(os.environ.get("KPH", "9"))  # debug phase limit


class _EarlyOut(Exception):
    pass


def _r(ap):
    return ap.bitcast(F32R)


def _ap(t, extra, dims):
    """Custom access pattern into a pool tile (offset-aware)."""
    a = t[:]
    return AP(a.tensor, a.offset + extra, dims)


def _hap(h, extra, dims):
    """Custom access pattern into a raw DRAM handle."""
    return AP(h, extra, dims)


def _ksl(h, kd, width):
    """[128, width] row k-tile of a [1024, width] DRAM weight."""
    return h.ap()[kd * 128:(kd + 1) * 128, :]


def build(rep=1):
    nc = bacc.Bacc("TRN2", target_bir_lowering=False, debug=False,
                   num_devices=NCORES)

    x_h = nc.dram_tensor("x", [NSP, D], F32, kind="ExternalInput")
    shapes = {"W1": [D, AD], "b1": [AD], "W2": [AD, D], "b2": [D],
              "Wc": [D, C], "bc": [C], "Wq": [D, A], "bq": [A],
              "Wv": [D, D], "bv": [D], "ln_g": [D], "ln_b": [D],
              "conv_w": [C, C, D], "conv_b": [C]}
    w_h = {k: nc.dram_tensor(k, shapes[k], F32, kind="ExternalInput")
           for k in WNAMES}
    score_h = nc.dram_tensor("score", [NS], F32, kind="ExternalOutput")
    logits_h = nc.dram_tensor("logits", [C], F32, kind="ExternalOutput")

    with tile.TileContext(nc) as tc:
        for r in range(rep):
            if r:
                tc.strict_bb_all_engine_barrier()
            _body(nc, tc, x_h, w_h, score_h, logits_h)
    nc.compile()
    return nc


def _copy(eng, nc, out, in_):
    if eng == "s":
        nc.scalar.copy(out, in_)
    else:
        nc.vector.tensor_copy(out, in_)


def _body(nc, tc, x_h, w_h, score_h, logits_h):
    import contextlib
    es = contextlib.ExitStack()
    P = es.enter_context(tc.tile_pool(name="persist", bufs=1))
    DP = es.enter_context(tc.tile_pool(name="dram", bufs=1, space="DRAM"))

    # ---------------- persistent SBUF ----------------
    w1b = P.tile([128, 8 * AD], BF16, tag="w1b")
    wvp = P.tile([128, 2 * D], BF16, tag="wvp")
    wqp = P.tile([128, 2 * A], BF16, tag="wqp")
    wcp = P.tile([128, 2 * C], BF16, tag="wcp")
    b1T = P.tile([128, 2], F32, tag="b1T")
    bqT = P.tile([128, 3], F32, tag="bqT")
    bcT = P.tile([C, 1], F32, tag="bcT")
    bvp_row = P.tile([1, D], F32, tag="bvp_row")
    b2T = P.tile([128, 8], F32, tag="b2T")
    hT = P.tile([128, 2 * NSP], BF16, tag="hT")
    qT = P.tile([128, 3 * NSP], BF16, tag="qT")
    logitsT = P.tile([C, NSP], F32, tag="logitsT")
    scoresT = P.tile([C, NSP], F32, tag="scoresT")
    snat = P.tile([128, C * NB], F32, tag="snat")
    wBb = P.tile([128, C * NB], BF16, tag="wBb")
    wsum_p = P.tile([128, C], F32, tag="wsum_p")
    tophT = P.tile([128, 2 * C], F32, tag="tophT")
    tophTb = P.tile([128, 2 * C], BF16, tag="tophTb")
    topqT = P.tile([128, 3 * C], BF16, tag="topqT")
    gmaxl = P.tile([C, 1], F32, tag="gmaxl")
    lmaxs = P.tile([C, 1], F32, tag="lmaxs")
    ib16 = P.tile([128, 128], BF16, tag="ib16")
    if32 = P.tile([128, 128], F32, tag="if32")
    ones = P.tile([128, 1], F32, tag="ones")
    padw = P.tile([128, C * NB], F32, tag="padw")

    # ---------------- DRAM bounce tiles ----------------
    d_l_in = DP.tile([C], F32, tag="d_l_in")
    d_l_out = DP.tile([C], F32, tag="d_l_out")
    d_th_in = DP.tile([128, 2 * C], F32, tag="d_th_in")
    d_th_out = DP.tile([128, 2 * C], F32, tag="d_th_out")
    d_sm_in = DP.tile([C], F32, tag="d_sm_in")
    d_sm_out = DP.tile([C], F32, tag="d_sm_out")
    d_bag_in = DP.tile([C, D + 1], F32, tag="d_bag_in")
    d_bag_out = DP.tile([C, D + 1], F32, tag="d_bag_out")
    d_mask = DP.tile([C, NSP], F32, tag="d_mask")
    d_tiny = DP.tile([16], F32, tag="d_tiny")
    d_zero = DP.tile([64], F32, tag="d_zero")

    masks.make_identity(nc, ib16[:])
    masks.make_identity(nc, if32[:])
    nc.vector.memset(ones[:], 1.0)
    # pad-row mask: 1.0 everywhere, 0.0 on the 22 padded instances of the
    # last 128-block (partition range not writable by compute engines).
    nc.vector.memset(padw[:], 1.0)
    zrow = P.tile([1, 64], F32, tag="zrow")
    nc.vector.memset(zrow[:], 0.0)
    nc.gpsimd.dma_start(d_zero[:], zrow[:])
    nc.gpsimd.dma_start(
        _ap(padw, 106 * (C * NB) + (NB - 1) * C, [[C * NB, 22], [1, C]]),
        _ap(d_zero, 0, [[0, 22], [1, C]]))

    # ================= prologue: biases =================
    nc.sync.dma_start(b1T[:], _hap(w_h["b1"], 0, [[1, 128], [128, 2]]))
    bc_row = P.tile([1, C], F32, tag="bc_row")
    bq_row = P.tile([1, A], F32, tag="bq_row")
    bv_row = P.tile([1, D], F32, tag="bv_row")
    nc.sync.dma_start(bc_row[:], _hap(w_h["bc"], 0, [[0, 1], [1, C]]))
    nc.sync.dma_start(bq_row[:], _hap(w_h["bq"], 0, [[0, 1], [1, A]]))
    nc.sync.dma_start(bv_row[:], _hap(w_h["bv"], 0, [[0, 1], [1, D]]))
    nc.sync.dma_start(b2T[:], _hap(w_h["b2"], 0, [[1, 128], [128, 8]]))

    # ================= prologue: weight fold =================
    with tc.tile_pool(name="fold_keep", bufs=1) as FK:
        w2T = FK.tile([128, 8 * AD], BF16, tag="w2T")
        wvb = FK.tile([128, 8 * D], BF16, tag="wvb")
        wqb = FK.tile([128, 8 * A], BF16, tag="wqb")
        wcb = FK.tile([128, 8 * C], BF16, tag="wcb")
        b2Tb = FK.tile([128, 8], BF16, tag="b2Tb")
        nc.vector.tensor_copy(b2Tb[:], b2T[:])
        with tc.tile_pool(name="fold_sb", bufs=2) as FS:
            for kd in range(8):
                t = FS.tile([128, AD], F32, tag="w1in")
                nc.sync.dma_start(t[:], _ksl(w_h["W1"], kd, AD))
                nc.vector.tensor_copy(w1b[:, kd * AD:(kd + 1) * AD], t[:])
            with tc.tile_pool(name="fold_tp", bufs=4, space="PSUM") as FTP:
                for t2 in range(2):
                    w2in = FS.tile([128, D], F32, tag="w2in")
                    nc.sync.dma_start(
                        w2in[:], w_h["W2"].ap()[t2 * 128:(t2 + 1) * 128, :])
                    w2b = FS.tile([128, D], BF16, tag="w2b")
                    nc.vector.tensor_copy(w2b[:], w2in[:])
                    for kd in range(8):
                        pt = FTP.tile([128, 128], BF16, tag="tp")
                        nc.tensor.transpose(
                            pt[:], w2b[:, kd * 128:(kd + 1) * 128], ib16[:])
                        _copy("s" if kd % 2 == 0 else "v", nc,
                              w2T[:, kd * AD + t2 * 128: kd * AD + t2 * 128 + 128],
                              pt[:])
            for kd in range(8):
                wv_k = FS.tile([128, D], F32, tag="wv_k")
                wq_k = FS.tile([128, A], F32, tag="wq_k")
                wc_k = FS.tile([128, C], F32, tag="wc_k")
                nc.sync.dma_start(wv_k[:], _ksl(w_h["Wv"], kd, D))
                nc.sync.dma_start(wq_k[:], _ksl(w_h["Wq"], kd, A))
                nc.sync.dma_start(wc_k[:], _ksl(w_h["Wc"], kd, C))
                nc.vector.tensor_copy(wvb[:, kd * D:(kd + 1) * D], wv_k[:])
                nc.vector.tensor_copy(wqb[:, kd * A:(kd + 1) * A], wq_k[:])
                nc.vector.tensor_copy(wcb[:, kd * C:(kd + 1) * C], wc_k[:])
            with tc.tile_pool(name="fold_mm", bufs=1, space="PSUM") as FP:
                psv = [[FP.tile([128, 512], F32, tag=f"psv{m}{n2}", name=f"psv{m}{n2}")
                        for n2 in range(2)] for m in range(2)]
                psq = [FP.tile([128, A], F32, tag=f"psq{m}", name=f"psq{m}") for m in range(2)]
                psc = [FP.tile([128, C], F32, tag=f"psc{m}", name=f"psc{m}") for m in range(2)]
                for kd in range(8):
                    st, sp = kd == 0, kd == 7
                    for m in range(2):
                        lh = w2T[:, kd * AD + m * 128: kd * AD + m * 128 + 128]
                        for n2 in range(2):
                            nc.tensor.matmul(
                                psv[m][n2][:], lh,
                                wvb[:, kd * D + n2 * 512: kd * D + (n2 + 1) * 512],
                                start=st, stop=sp)
                        nc.tensor.matmul(psq[m][:], lh,
                                         wqb[:, kd * A:(kd + 1) * A],
                                         start=st, stop=sp)
                        nc.tensor.matmul(psc[m][:], lh,
                                         wcb[:, kd * C:(kd + 1) * C],
                                         start=st, stop=sp)
                for m in range(2):
                    for n2 in range(2):
                        nc.scalar.copy(
                            wvp[:, m * D + n2 * 512: m * D + (n2 + 1) * 512],
                            psv[m][n2][:])
                    nc.scalar.copy(wqp[:, m * A:(m + 1) * A], psq[m][:])
                    nc.scalar.copy(wcp[:, m * C:(m + 1) * C], psc[m][:])
            with tc.tile_pool(name="fold_b", bufs=1, space="PSUM") as FB:
                pbv = [FB.tile([1, 512], F32, tag=f"pbv{n2}", name=f"pbv{n2}") for n2 in range(2)]
                pbq = FB.tile([1, A], F32, tag="pbq")
                pbc = FB.tile([1, C], F32, tag="pbc")
                for kd in range(8):
                    st, sp = kd == 0, kd == 7
                    lh = b2Tb[:, kd:kd + 1]
                    for n2 in range(2):
                        nc.tensor.matmul(
                            pbv[n2][:], lh,
                            wvb[:, kd * D + n2 * 512: kd * D + (n2 + 1) * 512],
                            start=st, stop=sp)
                    nc.tensor.matmul(pbq[:], lh, wqb[:, kd * A:(kd + 1) * A],
                                     start=st, stop=sp)
                    nc.tensor.matmul(pbc[:], lh, wcb[:, kd * C:(kd + 1) * C],
                                     start=st, stop=sp)
                for n2 in range(2):
                    nc.vector.tensor_add(
                        bvp_row[:, n2 * 512:(n2 + 1) * 512], pbv[n2][:],
                        bv_row[:, n2 * 512:(n2 + 1) * 512])
                bqp_row = FK.tile([1, A], F32, tag="bqp_row")
                bcp_row = FK.tile([1, C], F32, tag="bcp_row")
                nc.vector.tensor_add(bqp_row[:], pbq[:], bq_row[:])
                nc.vector.tensor_add(bcp_row[:], pbc[:], bc_row[:])
                # bq' -> column layout [128, 3]; bc' -> [2, 1] via DRAM bounce
                nc.gpsimd.dma_start(_ap(d_tiny, 0, [[0, 1], [1, C]]), bcp_row[:])
                nc.gpsimd.dma_start(bcT[:], _ap(d_tiny, 0, [[1, C], [1, 1]]))
                nc.gpsimd.dma_start(_ap(d_mask, 0, [[0, 1], [1, A]]), bqp_row[:])
                nc.gpsimd.dma_start(bqT[:], _ap(d_mask, 0, [[1, 128], [128, 3]]))

    # ================= pass A: hT, qT, logitsT =================
    esA = contextlib.ExitStack()
    XP = esA.enter_context(tc.tile_pool(name="xa", bufs=4))
    XB = esA.enter_context(tc.tile_pool(name="xbf", bufs=8))
    XT = esA.enter_context(tc.tile_pool(name="xT", bufs=2))
    TP = esA.enter_context(tc.tile_pool(name="tpA", bufs=2, space="PSUM"))
    MA = esA.enter_context(tc.tile_pool(name="mmA", bufs=2, space="PSUM"))
    SA = esA.enter_context(tc.tile_pool(name="smA", bufs=1, space="PSUM"))

    def stage2(xTt, off, nx):
        for m in range(2):
            ph = MA.tile([128, 512], F32, tag="ph")
            for kd in range(8):
                nc.tensor.matmul(
                    ph[:, :nx],
                    w1b[:, kd * AD + m * 128: kd * AD + m * 128 + 128],
                    xTt[:, kd * nx: kd * nx + nx],
                    start=(kd == 0), stop=(kd == 7))
            nc.scalar.activation(hT[:, m * NSP + off: m * NSP + off + nx],
                                 ph[:, :nx], ACT.Relu,
                                 bias=b1T[:, m:m + 1], scale=1.0)
        for a in range(3):
            pq = MA.tile([128, 512], F32, tag="pq")
            for k in range(2):
                nc.tensor.matmul(
                    pq[:, :nx],
                    wqp[:, k * A + a * 128: k * A + a * 128 + 128],
                    hT[:, k * NSP + off: k * NSP + off + nx],
                    start=(k == 0), stop=(k == 1))
            nc.scalar.activation(qT[:, a * NSP + off: a * NSP + off + nx],
                                 pq[:, :nx], ACT.Identity,
                                 bias=bqT[:, a:a + 1], scale=1.0)
        pl = SA.tile([C, 512], F32, tag="pl")
        for k in range(2):
            nc.tensor.matmul(pl[:, :nx], wcp[:, k * C:(k + 1) * C],
                             hT[:, k * NSP + off: k * NSP + off + nx],
                             start=(k == 0), stop=(k == 1))
        nc.scalar.activation(logitsT[:, off:off + nx], pl[:, :nx],
                             ACT.Identity, bias=bcT[:], scale=1.0)

    prev = None
    for (off, nx) in CHUNKS:
        nblk = nx // 128
        xbs = []
        for b in range(nblk):
            xt = XP.tile([128, D], F32, tag="x")
            nc.sync.dma_start(
                xt[:], x_h.ap()[off + b * 128: off + (b + 1) * 128, :])
            xb = XB.tile([128, D], BF16, tag="xb")
            nc.vector.tensor_copy(xb[:], xt[:])
            xbs.append(xb)
        xTt = XT.tile([128, 8 * 512], BF16, tag="xT")
        nci = 0
        for kd in range(8):
            for b in range(nblk):
                pt = TP.tile([128, 128], BF16, tag="tp")
                nc.tensor.transpose(
                    pt[:], xbs[b][:, kd * 128:(kd + 1) * 128], ib16[:])
                _copy("s" if nci % 2 == 0 else "v", nc,
                      xTt[:, kd * nx + b * 128: kd * nx + (b + 1) * 128], pt[:])
                nci += 1
        if prev is not None:
            stage2(*prev)
        prev = (xTt, off, nx)
    stage2(*prev)
    esA.close()

    def _dummy_out():
        nc.sync.dma_start(score_h.ap()[:], logitsT[0:1, 0:NS])
        nc.sync.dma_start(logits_h.ap()[:], logitsT[0:2, 0:1])

    if KPH < 2:
        _dummy_out()
        es.close()
        return

    # ================= critical instance =================
    try:
      with tc.tile_pool(name="ext_sb", bufs=1) as ES, \
            tc.tile_pool(name="ext_ps", bufs=2, space="PSUM") as EP:
        nc.vector.memset(logitsT[:, NS:NSP], NEG)
        lmaxl = ES.tile([C, 1], F32, tag="lmaxl")
        nc.vector.reduce_max(lmaxl[:], logitsT[:], axis=AX.X)
        nc.gpsimd.dma_start(d_l_in[:], lmaxl[:])
        nc.gpsimd.collective_compute(
            "AllReduce", ALU.max, replica_groups=RG,
            ins=[d_l_in[:]], outs=[d_l_out[:]])
        nc.gpsimd.dma_start(gmaxl[:], _ap(d_l_out, 0, [[1, C], [1, 1]]))
        if KPH == 11:
            raise _EarlyOut()
        maskT = ES.tile([C, NSP], F32, tag="maskT")
        nc.vector.tensor_scalar(maskT[:], logitsT[:], gmaxl[:], None,
                                op0=ALU.is_equal)
        nc.sync.dma_start(d_mask[:], maskT[:])
        junk = ES.tile([128, NSP], BF16, tag="junk")
        for c in range(C):
            mb = ES.tile([128, NSP], F32, tag="mb")
            nc.sync.dma_start(mb[:], _ap(d_mask, c * NSP, [[0, 128], [1, NSP]]))
            for k in range(2):
                nc.vector.tensor_mul(junk[:], hT[:, k * NSP:(k + 1) * NSP], mb[:])
                nc.vector.tensor_reduce(
                    tophT[:, k * C + c: k * C + c + 1], junk[:],
                    axis=AX.X, op=ALU.add)
        if KPH == 12:
            raise _EarlyOut()
        nc.sync.dma_start(d_th_in[:], tophT[:])
        nc.gpsimd.collective_compute(
            "AllReduce", ALU.add, replica_groups=RG,
            ins=[d_th_in[:]], outs=[d_th_out[:]])
        nc.sync.dma_start(tophT[:], d_th_out[:])
        nc.vector.tensor_copy(tophTb[:], tophT[:])
        for a in range(3):
            pt = EP.tile([128, C], F32, tag="ptq")
            for k in range(2):
                nc.tensor.matmul(pt[:],
                                 wqp[:, k * A + a * 128: k * A + a * 128 + 128],
                                 tophTb[:, k * C:(k + 1) * C],
                                 start=(k == 0), stop=(k == 1))
            nc.scalar.activation(topqT[:, a * C:(a + 1) * C], pt[:],
                                 ACT.Identity, bias=bqT[:, a:a + 1], scale=1.0)
    except _EarlyOut:
        _dummy_out()
        es.close()
        return

    if KPH < 3:
        _dummy_out()
        es.close()
        return

    # ================= pass B1: scores =================
    with tc.tile_pool(name="b1_ps", bufs=2, space="PSUM") as BP, \
            tc.tile_pool(name="b1_ps2", bufs=4, space="PSUM") as BP2, \
            tc.tile_pool(name="b1_sb", bufs=1) as BS:
        for (off, nx) in CHUNKS:
            ps = BP.tile([C, 512], F32, tag="ps")
            for a in range(3):
                nc.tensor.matmul(ps[:, :nx], topqT[:, a * C:(a + 1) * C],
                                 qT[:, a * NSP + off: a * NSP + off + nx],
                                 start=(a == 0), stop=(a == 2))
            nc.scalar.copy(scoresT[:, off:off + nx], ps[:, :nx])
            for b in range(nx // 128):
                blk = off // 128 + b
                p2 = BP2.tile([128, C], F32, tag="p2")
                for a in range(3):
                    nc.tensor.matmul(
                        p2[:],
                        qT[:, a * NSP + blk * 128: a * NSP + (blk + 1) * 128],
                        topqT[:, a * C:(a + 1) * C],
                        start=(a == 0), stop=(a == 2))
                nc.scalar.copy(snat[:, blk * C:(blk + 1) * C], p2[:])
        nc.vector.memset(scoresT[:, NS:NSP], NEG)
        nc.sync.dma_start(score_h.ap()[:], scoresT[0:1, 0:NS])
        nc.vector.reduce_max(lmaxs[:], scoresT[:], axis=AX.X)
        # wB = exp((snat - local_max) / sqrt(A))
        nc.gpsimd.dma_start(d_sm_in[:], lmaxs[:])
        lrep = BS.tile([128, C * NB], F32, tag="lrep")
        nc.sync.dma_start(lrep[:], _ap(d_sm_in, 0, [[0, 128], [0, NB], [1, C]]))
        sd = BS.tile([128, C * NB], F32, tag="sd")
        nc.vector.tensor_tensor(sd[:], snat[:], lrep[:], op=ALU.subtract)
        wE = BS.tile([128, C * NB], F32, tag="wE")
        nc.scalar.activation(wE[:], sd[:], ACT.Exp, bias=0.0, scale=RSQA)
        nc.vector.tensor_mul(wBb[:], wE[:], padw[:])
        nc.vector.tensor_reduce(
            wsum_p[:], wBb[:].rearrange("p (b c) -> p c b", c=C),
            axis=AX.X, op=ALU.add)

    if KPH < 4:
        nc.sync.dma_start(logits_h.ap()[:], lmaxs[0:2, 0:1])
        es.close()
        return

    # ================= pass B2: v-GEMM fused with bag pooling ============
    with tc.tile_pool(name="b2_bag", bufs=1, space="PSUM") as BGP:
        pbag = [BGP.tile([C, 512], F32, tag=f"pbag{n2}", name=f"pbag{n2}") for n2 in range(2)]
        with tc.tile_pool(name="b2_ps", bufs=4, space="PSUM") as VP, \
                tc.tile_pool(name="b2_sb", bufs=4) as VS:
            pend = None
            for blk in range(NB):
                cur = []
                for n2 in range(2):
                    pv = VP.tile([128, 512], F32, tag="pv")
                    for k in range(2):
                        nc.tensor.matmul(
                            pv[:],
                            hT[:, k * NSP + blk * 128: k * NSP + (blk + 1) * 128],
                            wvp[:, k * D + n2 * 512: k * D + (n2 + 1) * 512],
                            start=(k == 0), stop=(k == 1))
                    vsb = VS.tile([128, 512], BF16, tag="vsb")
                    nc.scalar.copy(vsb[:], pv[:])
                    cur.append(vsb)
                if pend is not None:
                    for n2 in range(2):
                        nc.tensor.matmul(
                            pbag[n2][:],
                            wBb[:, pend[1] * C:(pend[1] + 1) * C],
                            pend[0][n2][:],
                            start=(pend[1] == 0), stop=(pend[1] == NB - 1))
                pend = (cur, blk)
            for n2 in range(2):
                nc.tensor.matmul(pbag[n2][:],
                                 wBb[:, pend[1] * C:(pend[1] + 1) * C],
                                 pend[0][n2][:],
                                 start=(pend[1] == 0), stop=(pend[1] == NB - 1))

        # ================= epilogue =================
        with tc.tile_pool(name="ep_sb", bufs=1) as S, \
                tc.tile_pool(name="ep_ps", bufs=1, space="PSUM") as EPP:
            nc.gpsimd.collective_compute(
                "AllReduce", ALU.max, replica_groups=RG,
                ins=[d_sm_in[:]], outs=[d_sm_out[:]])
            gmaxs = S.tile([C, 1], F32, tag="gmaxs")
            nc.gpsimd.dma_start(gmaxs[:], _ap(d_sm_out, 0, [[1, C], [1, 1]]))
            gam = S.tile([C, 1], F32, tag="gam")
            nc.vector.tensor_scalar(gam[:], lmaxs[:], gmaxs[:], None,
                                    op0=ALU.subtract)
            nc.scalar.activation(gam[:], gam[:], ACT.Exp, bias=0.0, scale=RSQA)
            pws = EPP.tile([1, C], F32, tag="pws")
            nc.tensor.matmul(pws[:], ones[:], wsum_p[:],
                             start=True, stop=True)
            ws_row = S.tile([1, C], F32, tag="ws_row")
            nc.scalar.copy(ws_row[:], pws[:])
            nc.gpsimd.dma_start(_ap(d_tiny, 4, [[0, 1], [1, C]]), ws_row[:])
            ws_col = S.tile([C, 1], F32, tag="ws_col")
            nc.gpsimd.dma_start(ws_col[:], _ap(d_tiny, 4, [[1, C], [1, 1]]))
            pack = S.tile([C, D + 1], F32, tag="pack")
            for n2 in range(2):
                nc.scalar.activation(pack[:, n2 * 512:(n2 + 1) * 512],
                                     pbag[n2][:], ACT.Copy, bias=0.0,
                                     scale=gam[:])
            nc.vector.tensor_mul(pack[:, D:D + 1], ws_col[:], gam[:])
            nc.sync.dma_start(d_bag_in[:], pack[:])
            nc.gpsimd.collective_compute(
                "AllReduce", ALU.add, replica_groups=RG,
                ins=[d_bag_in[:]], outs=[d_bag_out[:]])
            gbag = S.tile([C, D + 1], F32, tag="gbag")
            nc.sync.dma_start(gbag[:], d_bag_out[:])

            winv = S.tile([C, 1], F32, tag="winv")
            nc.vector.reciprocal(winv[:], gbag[:, D:D + 1])
            bagf = S.tile([C, D], F32, tag="bagf")
            nc.scalar.activation(bagf[:], gbag[:, 0:D], ACT.Copy, bias=0.0,
                                 scale=winv[:])
            bv2 = S.tile([C, D], F32, tag="bv2")
            nc.gpsimd.dma_start(_ap(d_mask, 0, [[0, 1], [1, D]]), bvp_row[:])
            nc.sync.dma_start(bv2[:], _ap(d_mask, 0, [[0, C], [1, D]]))
            nc.vector.tensor_add(bagf[:], bagf[:], bv2[:])
            # layernorm
            mu = S.tile([C, 1], F32, tag="mu")
            nc.vector.reduce_sum(mu[:], bagf[:], axis=AX.X)
            nc.vector.tensor_scalar_mul(mu[:], mu[:], 1.0 / D)
            xc = S.tile([C, D], F32, tag="xc")
            nc.vector.tensor_scalar(xc[:], bagf[:], mu[:], None,
                                    op0=ALU.subtract)
            sq = S.tile([C, D], F32, tag="sq")
            var = S.tile([C, 1], F32, tag="var")
            nc.vector.tensor_mul(sq[:], xc[:], xc[:])
            nc.vector.tensor_reduce(var[:], sq[:], axis=AX.X, op=ALU.add)
            nc.vector.tensor_scalar_mul(var[:], var[:], 1.0 / D)
            sdv = S.tile([C, 1], F32, tag="sdv")
            eps = S.tile([C, 1], F32, tag="eps")
            nc.vector.memset(eps[:], 1e-5)
            nc.scalar.activation(sdv[:], var[:], ACT.Sqrt, bias=eps[:], scale=1.0)
            rinv = S.tile([C, 1], F32, tag="rinv")
            nc.vector.reciprocal(rinv[:], sdv[:])
            xn = S.tile([C, D], F32, tag="xn")
            nc.scalar.activation(xn[:], xc[:], ACT.Copy, bias=0.0, scale=rinv[:])
            g2 = S.tile([C, D], F32, tag="g2")
            b2r = S.tile([C, D], F32, tag="b2r")
            nc.sync.dma_start(g2[:], _hap(w_h["ln_g"], 0, [[0, C], [1, D]]))
            nc.sync.dma_start(b2r[:], _hap(w_h["ln_b"], 0, [[0, C], [1, D]]))
            nc.vector.tensor_mul(xn[:], xn[:], g2[:])
            nc.vector.tensor_add(xn[:], xn[:], b2r[:])
            # conv contraction: blog[o] = sum_cd xn[c,d]*conv_w[o,c,d]
            cw = S.tile([C, 2 * D], F32, tag="cw")
            for o in range(C):
                nc.sync.dma_start(cw[:, o * D:(o + 1) * D],
                                  w_h["conv_w"].ap()[o, :, :])
            cv = S.tile([C, C], F32, tag="cv")
            sq2 = S.tile([C, D], F32, tag="sq2")
            for o in range(C):
                nc.vector.tensor_mul(sq2[:], xn[:], cw[:, o * D:(o + 1) * D])
                nc.vector.tensor_reduce(cv[:, o:o + 1], sq2[:],
                                        axis=AX.X, op=ALU.add)
            pcv = EPP.tile([1, C], F32, tag="pcv")
            nc.tensor.matmul(pcv[:], ones[0:C, 0:1], cv[:],
                             start=True, stop=True)
            blog_row = S.tile([1, C], F32, tag="blog_row")
            nc.scalar.copy(blog_row[:], pcv[:])
            nc.gpsimd.dma_start(_ap(d_tiny, 8, [[0, 1], [1, C]]), blog_row[:])
            blog = S.tile([C, 1], F32, tag="blog")
            nc.gpsimd.dma_start(blog[:], _ap(d_tiny, 8, [[1, C], [1, 1]]))
            cb = S.tile([C, 1], F32, tag="cb")
            nc.gpsimd.dma_start(cb[:], _hap(w_h["conv_b"], 0, [[1, C], [1, 1]]))
            nc.vector.tensor_add(blog[:], blog[:], cb[:])
            nc.vector.tensor_add(blog[:], blog[:], gmaxl[:])
            nc.scalar.activation(blog[:], blog[:], ACT.Copy, bias=0.0, scale=0.5)
            nc.gpsimd.dma_start(logits_h.ap()[:], blog[:])

    es.close()


_NC = None


def _get_nc():
    global _NC
    if _NC is None:
        _NC = build()
    return _NC


def _make_in_maps(inputs):
    x = np.asarray(inputs["x"], np.float32)
    w = {k: np.ascontiguousarray(np.asarray(inputs[k], np.float32))
         for k in WNAMES}
    in_maps = []
    for i in range(NCORES):
        xs = np.zeros((NSP, D), np.float32)
        xs[:NS] = x[i * NS:(i + 1) * NS]
        m = {"x": xs}
        m.update(w)
        in_maps.append(m)
    return in_maps


def _assemble(results):
    score = np.concatenate([results[i]["score"] for i in range(NCORES)])
    logits = np.asarray(results[0]["logits"], np.float32)
    return logits, score.astype(np.float32)


def kernel(**inputs):
    nc = _get_nc()
    res = run_bass_kernel_spmd(nc, _make_in_maps(inputs),
                               core_ids=list(range(NCORES)))
    return _assemble(res.results)


def run_traced(**inputs):
    nc = _get_nc()
    res = run_bass_kernel_spmd(nc, _make_in_maps(inputs),
                               core_ids=list(range(NCORES)), trace=True)
    return _assemble(res.results), res


# revision 27
# speedup vs baseline: 422.5602x; 1.2528x over previous
"""DSMIL bag-of-instances kernel for one TRN2 chip (8 NeuronCores).

Strategy:
  - Shard N=50000 instances across 8 cores (6250 rows each, padded to
    6272 = 49*128).
  - Fold W2 into the downstream weights on device: with h = relu(x@W1+b1),
    f@M = h@(W2@M) + b2@M + b_M for M in {Wv,Wq,Wc}. Cuts matmul FLOPs ~3x.
  - Flash-attention-style softmax: exp with the LOCAL max, rescaled after a
    cross-core max, so the v-GEMM fuses with bag pooling and v is never
    materialized.
  - Critical-instance extraction without gathers: all-reduce(max) of the
    instance logits, then every core reduces hT against the
    (logits == gmax) mask row; non-owners contribute zeros to the
    all-reduce(add).
  - 4 tiny all-reduces: logits-max [2], top-feature [128,4], scores-max [2],
    bag+denominator [2,1025].
  - bf16 operands + fp32 PSUM accumulation; fp32r for fp32 matmuls.
"""

import math
import os
import sys

import ml_dtypes
import numpy as np

for _p in ("/opt/trn_rl_repo",):
    if _p not in sys.path:
        sys.path.insert(0, _p)

import concourse.bacc as bacc
import concourse.bass as bass
import concourse.mybir as mybir
import concourse.tile as tile
from concourse import masks
from concourse.ap import AP
from concourse.bass_utils import run_bass_kernel_spmd

F32 = mybir.dt.float32
F32R = mybir.dt.float32r
BF16 = mybir.dt.bfloat16
ALU = mybir.AluOpType
ACT = mybir.ActivationFunctionType
AX = mybir.AxisListType

NCORES = 8
N = 50000
NS = N // NCORES          # 6250 rows per core
NSP = 6272                # padded to 49 * 128
NB = NSP // 128           # 49
D = 1024
AD = 256                  # adaptor dim -> 2 k-tiles
A = 384                   # attn dim    -> 3 a-tiles
C = 2
CHUNKS = [(i * 1024, 1024) for i in range(6)] + [(6144, 128)]
RSQA = 1.0 / math.sqrt(float(A))
NEG = -1.0e30
RG = [list(range(NCORES))]

WNAMES = ("W1", "b1", "W2", "b2", "Wc", "bc", "Wq", "bq", "Wv", "bv",
          "ln_g", "ln_b", "conv_w", "conv_b")
KPH = int(os.environ.get("KPH", "9"))  # debug phase limit
NOCC = bool(int(os.environ.get("NOCC", "0")))  # replace collectives with copies


class _EarlyOut(Exception):
    pass


def _r(ap):
    return ap.bitcast(F32R)


def _ap(t, extra, dims):
    """Custom access pattern into a pool tile (offset-aware)."""
    a = t[:]
    return AP(a.tensor, a.offset + extra, dims)


def _hap(h, extra, dims):
    """Custom access pattern into a raw DRAM handle."""
    return AP(h, extra, dims)


def _ksl(h, kd, width):
    """[128, width] row k-tile of a [1024, width] DRAM weight."""
    return h.ap()[kd * 128:(kd + 1) * 128, :]


def build(rep=1, num_devices=NCORES):
    nc = bacc.Bacc("TRN2", target_bir_lowering=False, debug=False,
                   num_devices=num_devices)

    x_h = nc.dram_tensor("x", [D, NSP], BF16, kind="ExternalInput")
    shapes = {"W1": [D, AD], "b1": [AD], "W2": [AD, D], "b2": [D],
              "Wc": [D, C], "bc": [C], "Wq": [D, A], "bq": [A],
              "Wv": [D, D], "bv": [D], "ln_g": [D], "ln_b": [D],
              "conv_w": [C, C, D], "conv_b": [C]}
    w_h = {k: nc.dram_tensor(k, shapes[k], F32, kind="ExternalInput")
           for k in WNAMES}
    score_h = nc.dram_tensor("score", [NS], F32, kind="ExternalOutput")
    logits_h = nc.dram_tensor("logits", [C], F32, kind="ExternalOutput")

    with tile.TileContext(nc) as tc:
        for r in range(rep):
            if r:
                tc.strict_bb_all_engine_barrier()
            _body(nc, tc, x_h, w_h, score_h, logits_h)
    nc.compile()
    return nc


def _cc(nc, op, tin, tout):
    if NOCC:
        nc.gpsimd.dma_start(tout[:], tin[:])
    else:
        nc.gpsimd.collective_compute("AllReduce", op, replica_groups=RG,
                                     ins=[tin[:]], outs=[tout[:]])


def _copy(eng, nc, out, in_):
    if eng == "s":
        nc.scalar.copy(out, in_)
    else:
        nc.vector.tensor_copy(out, in_)


def _body(nc, tc, x_h, w_h, score_h, logits_h):
    import contextlib
    es = contextlib.ExitStack()
    P = es.enter_context(tc.tile_pool(name="persist", bufs=1))
    DP = es.enter_context(tc.tile_pool(name="dram", bufs=1, space="DRAM"))

    # ---------------- persistent SBUF ----------------
    w1b = P.tile([128, 8 * AD], BF16, tag="w1b")
    wvp = P.tile([128, 2 * D], BF16, tag="wvp")
    wqp = P.tile([128, 2 * A], BF16, tag="wqp")
    wcp = P.tile([128, 2 * C], BF16, tag="wcp")
    b1T = P.tile([128, 2], F32, tag="b1T")
    bqT = P.tile([128, 3], F32, tag="bqT")
    bcT = P.tile([C, 1], F32, tag="bcT")
    bvp_row = P.tile([1, D], F32, tag="bvp_row")
    b2T = P.tile([128, 8], F32, tag="b2T")
    hT = P.tile([128, 2 * NSP], BF16, tag="hT")
    qT = P.tile([128, 3 * NSP], BF16, tag="qT")
    logitsT = P.tile([C, NSP], F32, tag="logitsT")
    scoresTpad = P.tile([128, NSP], F32, tag="scoresTpad")
    snat = P.tile([128, C * NB], F32, tag="snat")
    wBb = P.tile([128, C * NB], BF16, tag="wBb")
    wsum_p = P.tile([128, C], F32, tag="wsum_p")
    tophT = P.tile([128, 2 * C], F32, tag="tophT")
    tophTb = P.tile([128, 2 * C], BF16, tag="tophTb")
    topqT = P.tile([128, 3 * C], BF16, tag="topqT")
    gmaxl = P.tile([C, 1], F32, tag="gmaxl")
    lmaxs = P.tile([C, 1], F32, tag="lmaxs")
    ib16 = P.tile([128, 128], BF16, tag="ib16")
    if32 = P.tile([128, 128], F32, tag="if32")
    ones = P.tile([128, 1], F32, tag="ones")
    padw = P.tile([128, C * NB], F32, tag="padw")

    # ---------------- DRAM bounce tiles ----------------
    d_l_in = DP.tile([C], F32, tag="d_l_in")
    d_l_out = DP.tile([C], F32, tag="d_l_out")
    d_th_in = DP.tile([128, 2 * C], F32, tag="d_th_in")
    d_th_out = DP.tile([128, 2 * C], F32, tag="d_th_out")
    d_sm_in = DP.tile([C], F32, tag="d_sm_in")
    d_sm_out = DP.tile([C], F32, tag="d_sm_out")
    d_bag_in = DP.tile([C, D + 1], F32, tag="d_bag_in")
    d_bag_out = DP.tile([C, D + 1], F32, tag="d_bag_out")
    d_mask = DP.tile([C, NSP], F32, tag="d_mask")
    d_tiny = DP.tile([16], F32, tag="d_tiny")
    d_zero = DP.tile([64], F32, tag="d_zero")

    masks.make_identity(nc, ib16[:])
    masks.make_identity(nc, if32[:])
    nc.vector.memset(ones[:], 1.0)
    # pad-row mask: 1.0 everywhere, 0.0 on the 22 padded instances of the
    # last 128-block (partition range not writable by compute engines).
    nc.vector.memset(padw[:], 1.0)
    zrow = P.tile([1, 64], F32, tag="zrow")
    nc.vector.memset(zrow[:], 0.0)
    nc.gpsimd.dma_start(d_zero[:], zrow[:])
    nc.gpsimd.dma_start(
        _ap(padw, 106 * (C * NB) + (NB - 1) * C, [[C * NB, 22], [1, C]]),
        _ap(d_zero, 0, [[0, 22], [1, C]]))

    # ================= prologue: biases =================
    nc.sync.dma_start(b1T[:], _hap(w_h["b1"], 0, [[1, 128], [128, 2]]))
    bc_row = P.tile([1, C], F32, tag="bc_row")
    bq_row = P.tile([1, A], F32, tag="bq_row")
    bv_row = P.tile([1, D], F32, tag="bv_row")
    nc.sync.dma_start(bc_row[:], _hap(w_h["bc"], 0, [[0, 1], [1, C]]))
    nc.sync.dma_start(bq_row[:], _hap(w_h["bq"], 0, [[0, 1], [1, A]]))
    nc.sync.dma_start(bv_row[:], _hap(w_h["bv"], 0, [[0, 1], [1, D]]))
    nc.sync.dma_start(b2T[:], _hap(w_h["b2"], 0, [[1, 128], [128, 8]]))

    # ================= prologue: weight fold =================
    with tc.tile_pool(name="fold_keep", bufs=1) as FK:
        w2T = FK.tile([128, 8 * AD], BF16, tag="w2T")
        wvb = FK.tile([128, 8 * D], BF16, tag="wvb")
        wqb = FK.tile([128, 8 * A], BF16, tag="wqb")
        wcb = FK.tile([128, 8 * C], BF16, tag="wcb")
        b2Tb = FK.tile([128, 8], BF16, tag="b2Tb")
        nc.vector.tensor_copy(b2Tb[:], b2T[:])
        with tc.tile_pool(name="fold_sb", bufs=2) as FS:
            for kd in range(8):
                t = FS.tile([128, AD], F32, tag="w1in")
                nc.sync.dma_start(t[:], _ksl(w_h["W1"], kd, AD))
                nc.vector.tensor_copy(w1b[:, kd * AD:(kd + 1) * AD], t[:])
            with tc.tile_pool(name="fold_tp", bufs=4, space="PSUM") as FTP:
                for t2 in range(2):
                    w2in = FS.tile([128, D], F32, tag="w2in")
                    nc.sync.dma_start(
                        w2in[:], w_h["W2"].ap()[t2 * 128:(t2 + 1) * 128, :])
                    w2b = FS.tile([128, D], BF16, tag="w2b")
                    nc.vector.tensor_copy(w2b[:], w2in[:])
                    for kd in range(8):
                        pt = FTP.tile([128, 128], BF16, tag="tp")
                        nc.tensor.transpose(
                            pt[:], w2b[:, kd * 128:(kd + 1) * 128], ib16[:])
                        _copy("s" if kd % 2 == 0 else "v", nc,
                              w2T[:, kd * AD + t2 * 128: kd * AD + t2 * 128 + 128],
                              pt[:])
            for kd in range(8):
                wv_k = FS.tile([128, D], F32, tag="wv_k")
                wq_k = FS.tile([128, A], F32, tag="wq_k")
                wc_k = FS.tile([128, C], F32, tag="wc_k")
                nc.sync.dma_start(wv_k[:], _ksl(w_h["Wv"], kd, D))
                nc.sync.dma_start(wq_k[:], _ksl(w_h["Wq"], kd, A))
                nc.sync.dma_start(wc_k[:], _ksl(w_h["Wc"], kd, C))
                nc.vector.tensor_copy(wvb[:, kd * D:(kd + 1) * D], wv_k[:])
                nc.vector.tensor_copy(wqb[:, kd * A:(kd + 1) * A], wq_k[:])
                nc.vector.tensor_copy(wcb[:, kd * C:(kd + 1) * C], wc_k[:])
            with tc.tile_pool(name="fold_mm", bufs=1, space="PSUM") as FP:
                psv = [[FP.tile([128, 512], F32, tag=f"psv{m}{n2}", name=f"psv{m}{n2}")
                        for n2 in range(2)] for m in range(2)]
                psq = [FP.tile([128, A], F32, tag=f"psq{m}", name=f"psq{m}") for m in range(2)]
                psc = [FP.tile([128, C], F32, tag=f"psc{m}", name=f"psc{m}") for m in range(2)]
                for kd in range(8):
                    st, sp = kd == 0, kd == 7
                    for m in range(2):
                        lh = w2T[:, kd * AD + m * 128: kd * AD + m * 128 + 128]
                        for n2 in range(2):
                            nc.tensor.matmul(
                                psv[m][n2][:], lh,
                                wvb[:, kd * D + n2 * 512: kd * D + (n2 + 1) * 512],
                                start=st, stop=sp)
                        nc.tensor.matmul(psq[m][:], lh,
                                         wqb[:, kd * A:(kd + 1) * A],
                                         start=st, stop=sp)
                        nc.tensor.matmul(psc[m][:], lh,
                                         wcb[:, kd * C:(kd + 1) * C],
                                         start=st, stop=sp)
                for m in range(2):
                    for n2 in range(2):
                        nc.scalar.copy(
                            wvp[:, m * D + n2 * 512: m * D + (n2 + 1) * 512],
                            psv[m][n2][:])
                    nc.scalar.copy(wqp[:, m * A:(m + 1) * A], psq[m][:])
                    nc.scalar.copy(wcp[:, m * C:(m + 1) * C], psc[m][:])
            with tc.tile_pool(name="fold_b", bufs=1, space="PSUM") as FB:
                pbv = [FB.tile([1, 512], F32, tag=f"pbv{n2}", name=f"pbv{n2}") for n2 in range(2)]
                pbq = FB.tile([1, A], F32, tag="pbq")
                pbc = FB.tile([1, C], F32, tag="pbc")
                for kd in range(8):
                    st, sp = kd == 0, kd == 7
                    lh = b2Tb[:, kd:kd + 1]
                    for n2 in range(2):
                        nc.tensor.matmul(
                            pbv[n2][:], lh,
                            wvb[:, kd * D + n2 * 512: kd * D + (n2 + 1) * 512],
                            start=st, stop=sp)
                    nc.tensor.matmul(pbq[:], lh, wqb[:, kd * A:(kd + 1) * A],
                                     start=st, stop=sp)
                    nc.tensor.matmul(pbc[:], lh, wcb[:, kd * C:(kd + 1) * C],
                                     start=st, stop=sp)
                for n2 in range(2):
                    nc.vector.tensor_add(
                        bvp_row[:, n2 * 512:(n2 + 1) * 512], pbv[n2][:],
                        bv_row[:, n2 * 512:(n2 + 1) * 512])
                bqp_row = FK.tile([1, A], F32, tag="bqp_row")
                bcp_row = FK.tile([1, C], F32, tag="bcp_row")
                nc.vector.tensor_add(bqp_row[:], pbq[:], bq_row[:])
                nc.vector.tensor_add(bcp_row[:], pbc[:], bc_row[:])
                # bq' -> column layout [128, 3]; bc' -> [2, 1] via DRAM bounce
                nc.gpsimd.dma_start(_ap(d_tiny, 0, [[0, 1], [1, C]]), bcp_row[:])
                nc.gpsimd.dma_start(bcT[:], _ap(d_tiny, 0, [[1, C], [1, 1]]))
                nc.gpsimd.dma_start(_ap(d_mask, 0, [[0, 1], [1, A]]), bqp_row[:])
                nc.gpsimd.dma_start(bqT[:], _ap(d_mask, 0, [[1, 128], [128, 3]]))

    # ================= pass A: hT, qT, logitsT =================
    esA = contextlib.ExitStack()
    XP = esA.enter_context(tc.tile_pool(name="xa", bufs=16))
    MA = esA.enter_context(tc.tile_pool(name="mmA", bufs=2, space="PSUM"))
    SA = esA.enter_context(tc.tile_pool(name="smA", bufs=2, space="PSUM"))

    for (off, nx) in CHUNKS:
        xts = []
        for kd in range(8):
            xt = XP.tile([128, 1024], BF16, tag="x")
            nc.sync.dma_start(xt[:, :nx],
                              x_h.ap()[kd * 128:(kd + 1) * 128, off:off + nx])
            xts.append(xt)
        nsl = [(0, 512), (512, 512)] if nx == 1024 else [(0, 128)]
        for m in range(2):
            for (no, nn) in nsl:
                ph = MA.tile([128, 512], F32, tag="ph")
                for kd in range(8):
                    nc.tensor.matmul(
                        ph[:, :nn],
                        w1b[:, kd * AD + m * 128: kd * AD + m * 128 + 128],
                        xts[kd][:, no:no + nn],
                        start=(kd == 0), stop=(kd == 7))
                nc.scalar.activation(
                    hT[:, m * NSP + off + no: m * NSP + off + no + nn],
                    ph[:, :nn], ACT.Relu, bias=b1T[:, m:m + 1], scale=1.0)
        for a in range(3):
            for (no, nn) in nsl:
                pq = MA.tile([128, 512], F32, tag="pq")
                for k in range(2):
                    nc.tensor.matmul(
                        pq[:, :nn],
                        wqp[:, k * A + a * 128: k * A + a * 128 + 128],
                        hT[:, k * NSP + off + no: k * NSP + off + no + nn],
                        start=(k == 0), stop=(k == 1))
                nc.scalar.activation(
                    qT[:, a * NSP + off + no: a * NSP + off + no + nn],
                    pq[:, :nn], ACT.Identity, bias=bqT[:, a:a + 1], scale=1.0)
        for (no, nn) in nsl:
            pl = SA.tile([C, 512], F32, tag="pl")
            for k in range(2):
                nc.tensor.matmul(pl[:, :nn], wcp[:, k * C:(k + 1) * C],
                                 hT[:, k * NSP + off + no: k * NSP + off + no + nn],
                                 start=(k == 0), stop=(k == 1))
            nc.scalar.activation(logitsT[:, off + no:off + no + nn], pl[:, :nn],
                                 ACT.Identity, bias=bcT[:], scale=1.0)
    esA.close()

    def _dummy_out():
        nc.sync.dma_start(score_h.ap()[:], logitsT[0:1, 0:NS])
        nc.sync.dma_start(logits_h.ap()[:], logitsT[0:2, 0:1])

    if KPH < 2:
        _dummy_out()
        es.close()
        return

    # ================= critical instance =================
    try:
      with tc.tile_pool(name="ext_sb", bufs=1) as ES, \
            tc.tile_pool(name="ext_ps", bufs=2, space="PSUM") as EP:
        nc.vector.memset(logitsT[:, NS:NSP], NEG)
        m8 = ES.tile([C, 8], F32, tag="m8")
        i8 = ES.tile([C, 8], mybir.dt.uint32, tag="i8")
        nc.vector.max(m8[:], logitsT[:])
        nc.vector.max_index(i8[:], m8[:], logitsT[:])
        lmaxl = ES.tile([C, 1], F32, tag="lmaxl")
        nc.vector.tensor_copy(lmaxl[:], m8[:, 0:1])
        nc.gpsimd.dma_start(d_l_in[:], lmaxl[:])
        _cc(nc, ALU.max, d_l_in, d_l_out)
        nc.gpsimd.dma_start(gmaxl[:], _ap(d_l_out, 0, [[1, C], [1, 1]]))
        if KPH == 11:
            raise _EarlyOut()
        # winner flag and per-class local argmax registers
        flag = ES.tile([C, 1], F32, tag="flag")
        nc.vector.tensor_scalar(flag[:], lmaxl[:], gmaxl[:], None,
                                op0=ALU.is_equal)
        nc.gpsimd.dma_start(_ap(d_tiny, 0, [[0, 1], [1, C]]),
                            i8[:, 0:1].bitcast(F32))
        idxrow = ES.tile([1, C], mybir.dt.uint32, tag="idxrow")
        nc.gpsimd.dma_start(idxrow[:].bitcast(F32), _ap(d_tiny, 0, [[0, 1], [1, C]]))
        nc.gpsimd.dma_start(_ap(d_tiny, 4, [[0, 1], [1, C]]), flag[:])
        flag_row = ES.tile([1, C], F32, tag="flag_row")
        nc.gpsimd.dma_start(flag_row[:], _ap(d_tiny, 4, [[0, 1], [1, C]]))
        fb = ES.tile([128, C], F32, tag="fb")
        nc.gpsimd.partition_broadcast(fb[:], flag_row[:])
        # gather hT[:, idx] per class (class-major cols: c*2 + k), mask, reduce
        for c in range(C):
            reg = nc.vector.alloc_register(None)
            nc.vector.reg_load(reg, idxrow[0:1, c:c + 1])
            iv = nc.snap(reg, min_val=0, max_val=NSP - 1, donate=True)
            for k in range(2):
                nc.vector.tensor_copy(
                    tophT[:, c * 2 + k: c * 2 + k + 1],
                    hT[:, bass.ds(iv + k * NSP, 1)])
            nc.vector.tensor_scalar_mul(tophT[:, c * 2:(c + 1) * 2],
                                        tophT[:, c * 2:(c + 1) * 2],
                                        fb[:, c:c + 1])
        if KPH == 12:
            raise _EarlyOut()
        nc.sync.dma_start(d_th_in[:], tophT[:])
        _cc(nc, ALU.add, d_th_in, d_th_out)
        nc.sync.dma_start(tophT[:], d_th_out[:])
        nc.vector.tensor_copy(tophTb[:], tophT[:])
        for a in range(3):
            pt = EP.tile([128, C], F32, tag="ptq")
            for k in range(2):
                nc.tensor.matmul(pt[:],
                                 wqp[:, k * A + a * 128: k * A + a * 128 + 128],
                                 _ap(tophTb, k, [[2 * C, 128], [2, C]]),
                                 start=(k == 0), stop=(k == 1))
            nc.scalar.activation(topqT[:, a * C:(a + 1) * C], pt[:],
                                 ACT.Identity, bias=bqT[:, a:a + 1], scale=1.0)
    except _EarlyOut:
        _dummy_out()
        es.close()
        return

    if KPH < 3:
        _dummy_out()
        es.close()
        return

    # ================= pass B1: scores =================
    with tc.tile_pool(name="b1_ps", bufs=2, space="PSUM") as BP, \
            tc.tile_pool(name="b1_ps2", bufs=4, space="PSUM") as BP2, \
            tc.tile_pool(name="b1_sb", bufs=1) as BS:
        for (off, nx) in CHUNKS:
            nsl = [(0, 512), (512, 512)] if nx == 1024 else [(0, 128)]
            for (no, nn) in nsl:
                ps = BP.tile([C, 512], F32, tag="ps")
                for a in range(3):
                    nc.tensor.matmul(
                        ps[:, :nn], topqT[:, a * C:(a + 1) * C],
                        qT[:, a * NSP + off + no: a * NSP + off + no + nn],
                        start=(a == 0), stop=(a == 2))
                nc.scalar.copy(scoresTpad[0:C, off + no:off + no + nn],
                               ps[:, :nn])
        nc.vector.memset(scoresTpad[0:C, NS:NSP], NEG)
        nc.sync.dma_start(score_h.ap()[:], scoresTpad[0:1, 0:NS])
        nc.vector.reduce_max(lmaxs[:], scoresTpad[0:C, :], axis=AX.X)
        # snat (natural layout) via PE transpose of scoresTpad blocks
        for blk in range(NB):
            p2 = BP2.tile([128, 128], F32, tag="p2")
            nc.tensor.transpose(
                p2[:], scoresTpad[:, blk * 128:(blk + 1) * 128], if32[:])
            nc.scalar.copy(snat[:, blk * C:(blk + 1) * C], p2[:, 0:C])
        # wB = exp((snat - local_max) / sqrt(A))
        nc.gpsimd.dma_start(d_sm_in[:], lmaxs[:])
        lrep = BS.tile([128, C * NB], F32, tag="lrep")
        nc.sync.dma_start(lrep[:], _ap(d_sm_in, 0, [[0, 128], [0, NB], [1, C]]))
        sd = BS.tile([128, C * NB], F32, tag="sd")
        nc.vector.tensor_tensor(sd[:], snat[:], lrep[:], op=ALU.subtract)
        wE = BS.tile([128, C * NB], F32, tag="wE")
        nc.scalar.activation(wE[:], sd[:], ACT.Exp, bias=0.0, scale=RSQA)
        nc.vector.tensor_mul(wBb[:], wE[:], padw[:])
        nc.vector.tensor_reduce(
            wsum_p[:], wBb[:].rearrange("p (b c) -> p c b", c=C),
            axis=AX.X, op=ALU.add)

    if KPH < 4:
        nc.sync.dma_start(logits_h.ap()[:], lmaxs[0:2, 0:1])
        es.close()
        return

    # ================= pass B2: v-GEMM fused with bag pooling ============
    with tc.tile_pool(name="b2_bag", bufs=1, space="PSUM") as BGP:
        pbag = [BGP.tile([C, 512], F32, tag=f"pbag{n2}", name=f"pbag{n2}") for n2 in range(2)]
        with tc.tile_pool(name="b2_ps", bufs=4, space="PSUM") as VP, \
                tc.tile_pool(name="b2_sb", bufs=4) as VS:
            pend = None
            for blk in range(NB):
                cur = []
                for n2 in range(2):
                    pv = VP.tile([128, 512], F32, tag="pv")
                    for k in range(2):
                        nc.tensor.matmul(
                            pv[:],
                            hT[:, k * NSP + blk * 128: k * NSP + (blk + 1) * 128],
                            wvp[:, k * D + n2 * 512: k * D + (n2 + 1) * 512],
                            start=(k == 0), stop=(k == 1))
                    vsb = VS.tile([128, 512], BF16, tag="vsb")
                    nc.scalar.copy(vsb[:], pv[:])
                    cur.append(vsb)
                if pend is not None:
                    for n2 in range(2):
                        nc.tensor.matmul(
                            pbag[n2][:],
                            wBb[:, pend[1] * C:(pend[1] + 1) * C],
                            pend[0][n2][:],
                            start=(pend[1] == 0), stop=(pend[1] == NB - 1))
                pend = (cur, blk)
            for n2 in range(2):
                nc.tensor.matmul(pbag[n2][:],
                                 wBb[:, pend[1] * C:(pend[1] + 1) * C],
                                 pend[0][n2][:],
                                 start=(pend[1] == 0), stop=(pend[1] == NB - 1))

        # ================= epilogue =================
        with tc.tile_pool(name="ep_sb", bufs=1) as S, \
                tc.tile_pool(name="ep_ps", bufs=1, space="PSUM") as EPP:
            _cc(nc, ALU.max, d_sm_in, d_sm_out)
            gmaxs = S.tile([C, 1], F32, tag="gmaxs")
            nc.gpsimd.dma_start(gmaxs[:], _ap(d_sm_out, 0, [[1, C], [1, 1]]))
            gam = S.tile([C, 1], F32, tag="gam")
            nc.vector.tensor_scalar(gam[:], lmaxs[:], gmaxs[:], None,
                                    op0=ALU.subtract)
            nc.scalar.activation(gam[:], gam[:], ACT.Exp, bias=0.0, scale=RSQA)
            pws = EPP.tile([1, C], F32, tag="pws")
            nc.tensor.matmul(pws[:], ones[:], wsum_p[:],
                             start=True, stop=True)
            ws_row = S.tile([1, C], F32, tag="ws_row")
            nc.scalar.copy(ws_row[:], pws[:])
            nc.gpsimd.dma_start(_ap(d_tiny, 4, [[0, 1], [1, C]]), ws_row[:])
            ws_col = S.tile([C, 1], F32, tag="ws_col")
            nc.gpsimd.dma_start(ws_col[:], _ap(d_tiny, 4, [[1, C], [1, 1]]))
            pack = S.tile([C, D + 1], F32, tag="pack")
            for n2 in range(2):
                nc.scalar.activation(pack[:, n2 * 512:(n2 + 1) * 512],
                                     pbag[n2][:], ACT.Copy, bias=0.0,
                                     scale=gam[:])
            nc.vector.tensor_mul(pack[:, D:D + 1], ws_col[:], gam[:])
            nc.sync.dma_start(d_bag_in[:], pack[:])
            _cc(nc, ALU.add, d_bag_in, d_bag_out)
            gbag = S.tile([C, D + 1], F32, tag="gbag")
            nc.sync.dma_start(gbag[:], d_bag_out[:])

            winv = S.tile([C, 1], F32, tag="winv")
            nc.vector.reciprocal(winv[:], gbag[:, D:D + 1])
            bagf = S.tile([C, D], F32, tag="bagf")
            nc.scalar.activation(bagf[:], gbag[:, 0:D], ACT.Copy, bias=0.0,
                                 scale=winv[:])
            bv2 = S.tile([C, D], F32, tag="bv2")
            nc.gpsimd.dma_start(_ap(d_mask, 0, [[0, 1], [1, D]]), bvp_row[:])
            nc.sync.dma_start(bv2[:], _ap(d_mask, 0, [[0, C], [1, D]]))
            nc.vector.tensor_add(bagf[:], bagf[:], bv2[:])
            # layernorm
            mu = S.tile([C, 1], F32, tag="mu")
            nc.vector.reduce_sum(mu[:], bagf[:], axis=AX.X)
            nc.vector.tensor_scalar_mul(mu[:], mu[:], 1.0 / D)
            xc = S.tile([C, D], F32, tag="xc")
            nc.vector.tensor_scalar(xc[:], bagf[:], mu[:], None,
                                    op0=ALU.subtract)
            sq = S.tile([C, D], F32, tag="sq")
            var = S.tile([C, 1], F32, tag="var")
            nc.vector.tensor_mul(sq[:], xc[:], xc[:])
            nc.vector.tensor_reduce(var[:], sq[:], axis=AX.X, op=ALU.add)
            nc.vector.tensor_scalar_mul(var[:], var[:], 1.0 / D)
            sdv = S.tile([C, 1], F32, tag="sdv")
            eps = S.tile([C, 1], F32, tag="eps")
            nc.vector.memset(eps[:], 1e-5)
            nc.scalar.activation(sdv[:], var[:], ACT.Sqrt, bias=eps[:], scale=1.0)
            rinv = S.tile([C, 1], F32, tag="rinv")
            nc.vector.reciprocal(rinv[:], sdv[:])
            xn = S.tile([C, D], F32, tag="xn")
            nc.scalar.activation(xn[:], xc[:], ACT.Copy, bias=0.0, scale=rinv[:])
            g2 = S.tile([C, D], F32, tag="g2")
            b2r = S.tile([C, D], F32, tag="b2r")
            nc.sync.dma_start(g2[:], _hap(w_h["ln_g"], 0, [[0, C], [1, D]]))
            nc.sync.dma_start(b2r[:], _hap(w_h["ln_b"], 0, [[0, C], [1, D]]))
            nc.vector.tensor_mul(xn[:], xn[:], g2[:])
            nc.vector.tensor_add(xn[:], xn[:], b2r[:])
            # conv contraction: blog[o] = sum_cd xn[c,d]*conv_w[o,c,d]
            cw = S.tile([C, 2 * D], F32, tag="cw")
            for o in range(C):
                nc.sync.dma_start(cw[:, o * D:(o + 1) * D],
                                  w_h["conv_w"].ap()[o, :, :])
            cv = S.tile([C, C], F32, tag="cv")
            sq2 = S.tile([C, D], F32, tag="sq2")
            for o in range(C):
                nc.vector.tensor_mul(sq2[:], xn[:], cw[:, o * D:(o + 1) * D])
                nc.vector.tensor_reduce(cv[:, o:o + 1], sq2[:],
                                        axis=AX.X, op=ALU.add)
            pcv = EPP.tile([1, C], F32, tag="pcv")
            nc.tensor.matmul(pcv[:], ones[0:C, 0:1], cv[:],
                             start=True, stop=True)
            blog_row = S.tile([1, C], F32, tag="blog_row")
            nc.scalar.copy(blog_row[:], pcv[:])
            nc.gpsimd.dma_start(_ap(d_tiny, 8, [[0, 1], [1, C]]), blog_row[:])
            blog = S.tile([C, 1], F32, tag="blog")
            nc.gpsimd.dma_start(blog[:], _ap(d_tiny, 8, [[1, C], [1, 1]]))
            cb = S.tile([C, 1], F32, tag="cb")
            nc.gpsimd.dma_start(cb[:], _hap(w_h["conv_b"], 0, [[1, C], [1, 1]]))
            nc.vector.tensor_add(blog[:], blog[:], cb[:])
            nc.vector.tensor_add(blog[:], blog[:], gmaxl[:])
            nc.scalar.activation(blog[:], blog[:], ACT.Copy, bias=0.0, scale=0.5)
            nc.gpsimd.dma_start(logits_h.ap()[:], blog[:])

    es.close()


_NC = None


def _get_nc():
    global _NC
    if _NC is None:
        _NC = build()
    return _NC


def _make_in_maps(inputs):
    x = np.asarray(inputs["x"], np.float32)
    w = {k: np.ascontiguousarray(np.asarray(inputs[k], np.float32))
         for k in WNAMES}
    in_maps = []
    xb = x.astype(ml_dtypes.bfloat16)
    for i in range(NCORES):
        xs = np.zeros((NSP, D), ml_dtypes.bfloat16)
        xs[:NS] = xb[i * NS:(i + 1) * NS]
        m = {"x": np.ascontiguousarray(xs.T)}
        m.update(w)
        in_maps.append(m)
    return in_maps


def _assemble(results):
    score = np.concatenate([results[i]["score"] for i in range(NCORES)])
    logits = np.asarray(results[0]["logits"], np.float32)
    return logits, score.astype(np.float32)


def kernel(**inputs):
    nc = _get_nc()
    res = run_bass_kernel_spmd(nc, _make_in_maps(inputs),
                               core_ids=list(range(NCORES)))
    return _assemble(res.results)


def run_traced(**inputs):
    nc = _get_nc()
    res = run_bass_kernel_spmd(nc, _make_in_maps(inputs),
                               core_ids=list(range(NCORES)), trace=True)
    return _assemble(res.results), res


# revision 39
# speedup vs baseline: 442.5825x; 1.0474x over previous
"""DSMIL bag-of-instances kernel for one TRN2 chip (8 NeuronCores).

Strategy:
  - Shard N=50000 instances across 8 cores (6250 rows each, padded to
    6272 = 49*128).
  - Fold W2 into the downstream weights on device: with h = relu(x@W1+b1),
    f@M = h@(W2@M) + b2@M + b_M for M in {Wv,Wq,Wc}. Cuts matmul FLOPs ~3x.
  - Flash-attention-style softmax: exp with the LOCAL max, rescaled after a
    cross-core max, so the v-GEMM fuses with bag pooling and v is never
    materialized.
  - Critical-instance extraction without gathers: all-reduce(max) of the
    instance logits, then every core reduces hT against the
    (logits == gmax) mask row; non-owners contribute zeros to the
    all-reduce(add).
  - 4 tiny all-reduces: logits-max [2], top-feature [128,4], scores-max [2],
    bag+denominator [2,1025].
  - bf16 operands + fp32 PSUM accumulation; fp32r for fp32 matmuls.
"""

import math
import os
import sys

import ml_dtypes
import numpy as np

for _p in ("/opt/trn_rl_repo",):
    if _p not in sys.path:
        sys.path.insert(0, _p)

import concourse.bacc as bacc
import concourse.bass as bass
import concourse.mybir as mybir
import concourse.tile as tile
from concourse import masks
from concourse.ap import AP
from concourse.bass_utils import run_bass_kernel_spmd

F32 = mybir.dt.float32
F32R = mybir.dt.float32r
BF16 = mybir.dt.bfloat16
ALU = mybir.AluOpType
ACT = mybir.ActivationFunctionType
AX = mybir.AxisListType

NCORES = 8
N = 50000
NS = N // NCORES          # 6250 rows per core
NSP = 6272                # padded to 49 * 128
NB = NSP // 128           # 49
D = 1024
AD = 256                  # adaptor dim -> 2 k-tiles
A = 384                   # attn dim    -> 3 a-tiles
C = 2
CHUNKS = [(i * 1024, 1024) for i in range(6)] + [(6144, 128)]
RSQA = 1.0 / math.sqrt(float(A))
NEG = -1.0e30
RG = [list(range(NCORES))]

WNAMES = ("W1", "b1", "W2", "b2", "Wc", "bc", "Wq", "bq", "Wv", "bv",
          "ln_g", "ln_b", "conv_w", "conv_b")
KPH = int(os.environ.get("KPH", "9"))  # debug phase limit
NOCC = bool(int(os.environ.get("NOCC", "0")))  # replace collectives with copies


class _EarlyOut(Exception):
    pass


def _r(ap):
    return ap.bitcast(F32R)


def _ap(t, extra, dims):
    """Custom access pattern into a pool tile (offset-aware)."""
    a = t[:]
    return AP(a.tensor, a.offset + extra, dims)


def _hap(h, extra, dims):
    """Custom access pattern into a raw DRAM handle."""
    return AP(h, extra, dims)


def _ksl(h, kd, width):
    """[128, width] row k-tile of a [1024, width] DRAM weight."""
    return h.ap()[kd * 128:(kd + 1) * 128, :]


def build(rep=1, num_devices=NCORES):
    nc = bacc.Bacc("TRN2", target_bir_lowering=False, debug=False,
                   num_devices=num_devices)

    x_h = nc.dram_tensor("x", [D, NSP], BF16, kind="ExternalInput")
    shapes = {"W1": [D, AD], "b1": [AD], "W2": [D, AD], "b2": [D],
              "Wc": [D, C], "bc": [C], "Wq": [D, A], "bq": [A],
              "Wv": [D, D], "bv": [D], "ln_g": [D], "ln_b": [D],
              "conv_w": [C, C, D], "conv_b": [C]}
    BF_W = {"W1", "W2", "Wc", "Wq", "Wv"}
    w_h = {k: nc.dram_tensor(k, shapes[k], BF16 if k in BF_W else F32,
                             kind="ExternalInput")
           for k in WNAMES}
    score_h = nc.dram_tensor("score", [NS], F32, kind="ExternalOutput")
    logits_h = nc.dram_tensor("logits", [C], F32, kind="ExternalOutput")

    with tile.TileContext(nc) as tc:
        for r in range(rep):
            if r:
                tc.strict_bb_all_engine_barrier()
            _body(nc, tc, x_h, w_h, score_h, logits_h)
    nc.compile()
    return nc


def _cc(nc, op, tin, tout):
    if NOCC:
        nc.gpsimd.dma_start(tout[:], tin[:])
    else:
        nc.gpsimd.collective_compute("AllReduce", op, replica_groups=RG,
                                     ins=[tin[:]], outs=[tout[:]])


def _copy(eng, nc, out, in_):
    if eng == "s":
        nc.scalar.copy(out, in_)
    else:
        nc.vector.tensor_copy(out, in_)


def _body(nc, tc, x_h, w_h, score_h, logits_h):
    import contextlib
    es = contextlib.ExitStack()
    P = es.enter_context(tc.tile_pool(name="persist", bufs=1))
    DP = es.enter_context(tc.tile_pool(name="dram", bufs=1, space="DRAM"))

    # ---------------- persistent SBUF ----------------
    w1b = P.tile([128, 8 * AD], BF16, tag="w1b")
    wvp = P.tile([128, 2 * D], F32R, tag="wvp")
    wqp = P.tile([128, 2 * A], BF16, tag="wqp")
    wcp = P.tile([128, 2 * C], BF16, tag="wcp")
    b1T = P.tile([128, 2], F32, tag="b1T")
    bqT = P.tile([128, 3], F32, tag="bqT")
    bcT = P.tile([C, 1], F32, tag="bcT")
    bvp_row = P.tile([1, D], F32, tag="bvp_row")
    b2T = P.tile([128, 8], F32, tag="b2T")
    hT = P.tile([128, 2 * NSP], BF16, tag="hT")
    qT = P.tile([128, 3 * NSP], BF16, tag="qT")
    logitsT = P.tile([C, NSP], F32, tag="logitsT")
    scoresTpad = P.tile([128, NSP], F32, tag="scoresTpad")
    snat = P.tile([128, C * NB], F32, tag="snat")
    wBb = P.tile([128, C * NB], BF16, tag="wBb")
    wsum_p = P.tile([128, C], F32, tag="wsum_p")
    tophT = P.tile([128, 2 * C], F32, tag="tophT")
    tophTb = P.tile([128, 2 * C], BF16, tag="tophTb")
    topqT = P.tile([128, 3 * C], BF16, tag="topqT")
    gmaxl = P.tile([C, 1], F32, tag="gmaxl")
    lmaxs = P.tile([C, 1], F32, tag="lmaxs")
    hN = P.tile([128, NB * AD], BF16, tag="hN")
    lacc = P.tile([C, 1], F32, tag="lacc")
    sacc = P.tile([C, 1], F32, tag="sacc")
    tred = P.tile([C, 1], F32, tag="tred")
    z8 = P.tile([C, 8], F32, tag="z8")
    ib16 = P.tile([128, 128], BF16, tag="ib16")
    if32 = P.tile([128, 128], F32, tag="if32")
    ones = P.tile([128, 1], F32, tag="ones")
    padw = P.tile([128, C * NB], F32, tag="padw")

    # ---------------- DRAM bounce tiles ----------------
    d_l_in = DP.tile([C], F32, tag="d_l_in")
    d_l_out = DP.tile([C], F32, tag="d_l_out")
    d_th_in = DP.tile([128, 2 * C], F32, tag="d_th_in")
    d_th_out = DP.tile([128, 2 * C], F32, tag="d_th_out")
    d_sm_in = DP.tile([C], F32, tag="d_sm_in")
    d_sm_out = DP.tile([C], F32, tag="d_sm_out")
    d_bag_in = DP.tile([C, D + 1], F32, tag="d_bag_in")
    d_bag_out = DP.tile([C, D + 1], F32, tag="d_bag_out")
    d_mask = DP.tile([C, NSP], F32, tag="d_mask")
    d_tiny = DP.tile([16], F32, tag="d_tiny")
    d_zero = DP.tile([64], F32, tag="d_zero")

    masks.make_identity(nc, ib16[:])
    masks.make_identity(nc, if32[:])
    nc.vector.memset(ones[:], 1.0)
    nc.vector.memset(z8[:], 0.0)
    nc.vector.memset(lacc[:], NEG)
    nc.vector.memset(sacc[:], NEG)
    # pad-row mask: 1.0 everywhere, 0.0 on the 22 padded instances of the
    # last 128-block (partition range not writable by compute engines).
    nc.vector.memset(padw[:], 1.0)
    zrow = P.tile([1, 64], F32, tag="zrow")
    nc.vector.memset(zrow[:], 0.0)
    nc.gpsimd.dma_start(d_zero[:], zrow[:])
    nc.gpsimd.dma_start(
        _ap(padw, 106 * (C * NB) + (NB - 1) * C, [[C * NB, 22], [1, C]]),
        _ap(d_zero, 0, [[0, 22], [1, C]]))

    # ================= prologue: biases =================
    nc.sync.dma_start(b1T[:], _hap(w_h["b1"], 0, [[1, 128], [128, 2]]))
    bc_row = P.tile([1, C], F32, tag="bc_row")
    bq_row = P.tile([1, A], F32, tag="bq_row")
    bv_row = P.tile([1, D], F32, tag="bv_row")
    nc.sync.dma_start(bc_row[:], _hap(w_h["bc"], 0, [[0, 1], [1, C]]))
    nc.sync.dma_start(bq_row[:], _hap(w_h["bq"], 0, [[0, 1], [1, A]]))
    nc.sync.dma_start(bv_row[:], _hap(w_h["bv"], 0, [[0, 1], [1, D]]))
    nc.sync.dma_start(b2T[:], _hap(w_h["b2"], 0, [[1, 128], [128, 8]]))

    # ================= pass A1: hT (G1) =================
    esA = contextlib.ExitStack()
    XP = esA.enter_context(tc.tile_pool(name="xa", bufs=12))
    MA = esA.enter_context(tc.tile_pool(name="mmA", bufs=2, space="PSUM"))
    SA = esA.enter_context(tc.tile_pool(name="smA", bufs=1, space="PSUM"))

    for kd in range(8):
        nc.sync.dma_start(w1b[:, kd * AD:(kd + 1) * AD], _ksl(w_h["W1"], kd, AD))
    for (off, nx) in CHUNKS:
        xts = []
        for kd in range(8):
            xt = XP.tile([128, 1024], BF16, tag="x")
            nc.sync.dma_start(xt[:, :nx],
                              x_h.ap()[kd * 128:(kd + 1) * 128, off:off + nx])
            xts.append(xt)
        nsl = [(0, 512), (512, 512)] if nx == 1024 else [(0, 128)]
        for m in range(2):
            for (no, nn) in nsl:
                ph = MA.tile([128, 512], F32, tag="ph")
                for kd in range(8):
                    nc.tensor.matmul(
                        ph[:, :nn],
                        w1b[:, kd * AD + m * 128: kd * AD + m * 128 + 128],
                        xts[kd][:, no:no + nn],
                        start=(kd == 0), stop=(kd == 7))
                nc.scalar.activation(
                    hT[:, m * NSP + off + no: m * NSP + off + no + nn],
                    ph[:, :nn], ACT.Relu, bias=b1T[:, m:m + 1], scale=1.0)
    esA.close()

    # ================= prologue: weight fold =================
    # (W1/W2T/Wv/Wq/Wc arrive bf16; W2 pre-transposed on host)
    with tc.tile_pool(name="fold_keep", bufs=1) as FK:
        w2T = FK.tile([128, 8 * AD], BF16, tag="w2T")
        wvb = FK.tile([128, 8 * D], BF16, tag="wvb")
        wqb = FK.tile([128, 8 * A], BF16, tag="wqb")
        wcb = FK.tile([128, 8 * C], BF16, tag="wcb")
        b2Tb = FK.tile([128, 8], BF16, tag="b2Tb")
        nc.vector.tensor_copy(b2Tb[:], b2T[:])
        for kd in range(8):
            nc.sync.dma_start(w2T[:, kd * AD:(kd + 1) * AD],
                              _ksl(w_h["W2"], kd, AD))
            nc.sync.dma_start(wvb[:, kd * D:(kd + 1) * D],
                              _ksl(w_h["Wv"], kd, D))
            nc.sync.dma_start(wqb[:, kd * A:(kd + 1) * A],
                              _ksl(w_h["Wq"], kd, A))
            nc.sync.dma_start(wcb[:, kd * C:(kd + 1) * C],
                              _ksl(w_h["Wc"], kd, C))
        with tc.tile_pool(name="fold_mm", bufs=1, space="PSUM") as FP:
            psv = [FP.tile([128, 512], F32, tag=f"psvn{n2}", name=f"psvn{n2}")
                   for n2 in range(2)]
            psq = FP.tile([128, A], F32, tag="psq")
            psc = FP.tile([128, C], F32, tag="psc")
            for m in range(2):
                for kd in range(8):
                    st, sp = kd == 0, kd == 7
                    lh = w2T[:, kd * AD + m * 128: kd * AD + m * 128 + 128]
                    for n2 in range(2):
                        nc.tensor.matmul(
                            psv[n2][:], lh,
                            wvb[:, kd * D + n2 * 512: kd * D + (n2 + 1) * 512],
                            start=st, stop=sp)
                    nc.tensor.matmul(psq[:], lh,
                                     wqb[:, kd * A:(kd + 1) * A],
                                     start=st, stop=sp)
                    nc.tensor.matmul(psc[:], lh,
                                     wcb[:, kd * C:(kd + 1) * C],
                                     start=st, stop=sp)
                for n2 in range(2):
                    nc.scalar.copy(
                        wvp[:, m * D + n2 * 512: m * D + (n2 + 1) * 512],
                        psv[n2][:])
                nc.scalar.copy(wqp[:, m * A:(m + 1) * A], psq[:])
                nc.scalar.copy(wcp[:, m * C:(m + 1) * C], psc[:])
        with tc.tile_pool(name="fold_b1", bufs=1, space="PSUM") as FB:
            pbv = [FB.tile([1, 512], F32, tag=f"pbv{n2}", name=f"pbv{n2}") for n2 in range(2)]
            for kd in range(8):
                st, sp = kd == 0, kd == 7
                lh = b2Tb[:, kd:kd + 1]
                for n2 in range(2):
                    nc.tensor.matmul(
                        pbv[n2][:], lh,
                        wvb[:, kd * D + n2 * 512: kd * D + (n2 + 1) * 512],
                        start=st, stop=sp)
            for n2 in range(2):
                nc.vector.tensor_add(
                    bvp_row[:, n2 * 512:(n2 + 1) * 512], pbv[n2][:],
                    bv_row[:, n2 * 512:(n2 + 1) * 512])
        with tc.tile_pool(name="fold_b2", bufs=1, space="PSUM") as FB:
            pbq = FB.tile([1, A], F32, tag="pbq")
            pbc = FB.tile([1, C], F32, tag="pbc")
            for kd in range(8):
                st, sp = kd == 0, kd == 7
                lh = b2Tb[:, kd:kd + 1]
                nc.tensor.matmul(pbq[:], lh, wqb[:, kd * A:(kd + 1) * A],
                                 start=st, stop=sp)
                nc.tensor.matmul(pbc[:], lh, wcb[:, kd * C:(kd + 1) * C],
                                 start=st, stop=sp)
            bqp_row = FK.tile([1, A], F32, tag="bqp_row")
            bqp_row = FK.tile([1, A], F32, tag="bqp_row")
            bcp_row = FK.tile([1, C], F32, tag="bcp_row")
            nc.vector.tensor_add(bqp_row[:], pbq[:], bq_row[:])
            nc.vector.tensor_add(bcp_row[:], pbc[:], bc_row[:])
            nc.gpsimd.dma_start(_ap(d_tiny, 0, [[0, 1], [1, C]]), bcp_row[:])
            nc.gpsimd.dma_start(bcT[:], _ap(d_tiny, 0, [[1, C], [1, 1]]))
            nc.gpsimd.dma_start(_ap(d_mask, 0, [[0, 1], [1, A]]), bqp_row[:])
            nc.gpsimd.dma_start(bqT[:], _ap(d_mask, 0, [[1, 128], [128, 3]]))

    # epilogue constants (loaded early so the tail doesn't wait on DMA)
    g2 = P.tile([C, D], F32, tag="g2")
    b2r = P.tile([C, D], F32, tag="b2r")
    cw = P.tile([C, 2 * D], F32, tag="cw")
    cb = P.tile([C, 1], F32, tag="cb")
    nc.sync.dma_start(g2[:], _hap(w_h["ln_g"], 0, [[0, C], [1, D]]))
    nc.sync.dma_start(b2r[:], _hap(w_h["ln_b"], 0, [[0, C], [1, D]]))
    for o in range(C):
        nc.sync.dma_start(cw[:, o * D:(o + 1) * D], w_h["conv_w"].ap()[o, :, :])
    nc.gpsimd.dma_start(cb[:], _hap(w_h["conv_b"], 0, [[1, C], [1, 1]]))


    # ================= pass A2: qT, logitsT =================
    esA = contextlib.ExitStack()
    MA = esA.enter_context(tc.tile_pool(name="mmA2", bufs=2, space="PSUM"))
    SA = esA.enter_context(tc.tile_pool(name="smA2", bufs=1, space="PSUM"))
    for (off, nx) in CHUNKS:
        nsl = [(0, 512), (512, 512)] if nx == 1024 else [(0, 128)]
        for a in range(3):
            for (no, nn) in nsl:
                pq = MA.tile([128, 512], F32, tag="pq")
                for k in range(2):
                    nc.tensor.matmul(
                        pq[:, :nn],
                        wqp[:, k * A + a * 128: k * A + a * 128 + 128],
                        hT[:, k * NSP + off + no: k * NSP + off + no + nn],
                        start=(k == 0), stop=(k == 1))
                nc.scalar.activation(
                    qT[:, a * NSP + off + no: a * NSP + off + no + nn],
                    pq[:, :nn], ACT.Identity, bias=bqT[:, a:a + 1], scale=1.0)
        for (no, nn) in nsl:
            pl = SA.tile([C, 512], F32, tag="pl")
            for k in range(2):
                nc.tensor.matmul(pl[:, :nn], wcp[:, k * C:(k + 1) * C],
                                 hT[:, k * NSP + off + no: k * NSP + off + no + nn],
                                 start=(k == 0), stop=(k == 1))
            nc.scalar.activation(logitsT[:, off + no:off + no + nn], pl[:, :nn],
                                 ACT.Identity, bias=bcT[:], scale=1.0)
            if off + no < NS:  # padded tail columns excluded via NS clamp
                hi = min(off + no + nn, NS)
                nc.vector.tensor_reduce(tred[:], logitsT[:, off + no:hi],
                                        axis=AX.X, op=ALU.max)
                nc.vector.tensor_max(lacc[:], lacc[:], tred[:])
    esA.close()

    # h_nat: transpose hT blocks once; reused by the wh contraction in B2
    with tc.tile_pool(name="hn_ps", bufs=4, space="PSUM") as HP:
        for blk in range(NB):
            for k in range(2):
                pt = HP.tile([128, 128], BF16, tag="pt")
                nc.tensor.transpose(
                    pt[:], hT[:, k * NSP + blk * 128: k * NSP + (blk + 1) * 128],
                    ib16[:])
                nc.vector.tensor_copy(hN[:, blk * AD + k * 128: blk * AD + (k + 1) * 128],
                                      pt[:])

    def _dummy_out():
        nc.sync.dma_start(score_h.ap()[:], logitsT[0:1, 0:NS])
        nc.sync.dma_start(logits_h.ap()[:], logitsT[0:2, 0:1])

    if KPH < 2:
        _dummy_out()
        es.close()
        return

    # ================= critical instance =================
    try:
      with tc.tile_pool(name="ext_sb", bufs=1) as ES, \
            tc.tile_pool(name="ext_ps", bufs=2, space="PSUM") as EP:
        nc.vector.memset(logitsT[:, NS:NSP], NEG)
        lmaxl = ES.tile([C, 1], F32, tag="lmaxl")
        nc.vector.tensor_copy(lmaxl[:], lacc[:])
        nc.gpsimd.dma_start(d_l_in[:], lmaxl[:])
        _cc(nc, ALU.max, d_l_in, d_l_out)
        # local argmax (runs concurrently with the collective)
        m8 = ES.tile([C, 8], F32, tag="m8")
        i8 = ES.tile([C, 8], mybir.dt.uint32, tag="i8")
        nc.vector.tensor_scalar(m8[:], z8[:], lmaxl[:], None, op0=ALU.add)
        nc.vector.max_index(i8[:], m8[:], logitsT[:])
        nc.gpsimd.dma_start(_ap(d_tiny, 0, [[0, 1], [1, C]]),
                            i8[:, 0:1].bitcast(F32))
        idxrow = ES.tile([1, C], mybir.dt.uint32, tag="idxrow")
        nc.gpsimd.dma_start(idxrow[:].bitcast(F32), _ap(d_tiny, 0, [[0, 1], [1, C]]))
        for c in range(C):
            reg = nc.vector.alloc_register(None)
            nc.vector.reg_load(reg, idxrow[0:1, c:c + 1])
            iv = nc.snap(reg, min_val=0, max_val=NSP - 1, donate=True)
            for k in range(2):
                nc.vector.tensor_copy(
                    tophT[:, c * 2 + k: c * 2 + k + 1],
                    hT[:, bass.ds(iv + k * NSP, 1)])
        nc.gpsimd.dma_start(gmaxl[:], _ap(d_l_out, 0, [[1, C], [1, 1]]))
        if KPH == 11:
            raise _EarlyOut()
        flag = ES.tile([C, 1], F32, tag="flag")
        nc.vector.tensor_scalar(flag[:], lmaxl[:], gmaxl[:], None,
                                op0=ALU.is_equal)
        nc.gpsimd.dma_start(_ap(d_tiny, 4, [[0, 1], [1, C]]), flag[:])
        flag_row = ES.tile([1, C], F32, tag="flag_row")
        nc.gpsimd.dma_start(flag_row[:], _ap(d_tiny, 4, [[0, 1], [1, C]]))
        fb = ES.tile([128, C], F32, tag="fb")
        nc.gpsimd.partition_broadcast(fb[:], flag_row[:])
        for c in range(C):
            nc.vector.tensor_scalar_mul(tophT[:, c * 2:(c + 1) * 2],
                                        tophT[:, c * 2:(c + 1) * 2],
                                        fb[:, c:c + 1])
        if KPH == 12:
            raise _EarlyOut()
        nc.sync.dma_start(d_th_in[:], tophT[:])
        _cc(nc, ALU.add, d_th_in, d_th_out)
        nc.sync.dma_start(tophT[:], d_th_out[:])
        nc.vector.tensor_copy(tophTb[:], tophT[:])
        for a in range(3):
            pt = EP.tile([128, C], F32, tag="ptq")
            for k in range(2):
                nc.tensor.matmul(pt[:],
                                 wqp[:, k * A + a * 128: k * A + a * 128 + 128],
                                 _ap(tophTb, k, [[2 * C, 128], [2, C]]),
                                 start=(k == 0), stop=(k == 1))
            nc.scalar.activation(topqT[:, a * C:(a + 1) * C], pt[:],
                                 ACT.Identity, bias=bqT[:, a:a + 1], scale=1.0)
    except _EarlyOut:
        _dummy_out()
        es.close()
        return

    if KPH < 3:
        _dummy_out()
        es.close()
        return

    # ================= pass B1: scores =================
    with tc.tile_pool(name="b1_ps", bufs=2, space="PSUM") as BP, \
            tc.tile_pool(name="b1_ps2", bufs=4, space="PSUM") as BP2, \
            tc.tile_pool(name="b1_sb", bufs=1) as BS:
        for (off, nx) in CHUNKS:
            nsl = [(0, 512), (512, 512)] if nx == 1024 else [(0, 128)]
            for (no, nn) in nsl:
                ps = BP.tile([C, 512], F32, tag="ps")
                for a in range(3):
                    nc.tensor.matmul(
                        ps[:, :nn], topqT[:, a * C:(a + 1) * C],
                        qT[:, a * NSP + off + no: a * NSP + off + no + nn],
                        start=(a == 0), stop=(a == 2))
                nc.scalar.copy(scoresTpad[0:C, off + no:off + no + nn],
                               ps[:, :nn])
                if off + no < NS:
                    hi = min(off + no + nn, NS)
                    nc.vector.tensor_reduce(tred[:],
                                            scoresTpad[0:C, off + no:hi],
                                            axis=AX.X, op=ALU.max)
                    nc.vector.tensor_max(sacc[:], sacc[:], tred[:])
        nc.vector.memset(scoresTpad[0:C, NS:NSP], NEG)
        nc.sync.dma_start(score_h.ap()[:], scoresTpad[0:1, 0:NS])
        nc.vector.tensor_copy(lmaxs[:], sacc[:])
        nc.gpsimd.dma_start(d_sm_in[:], lmaxs[:])
        _cc(nc, ALU.max, d_sm_in, d_sm_out)
        # snat (natural layout) via PE transpose of scoresTpad blocks
        for blk in range(NB):
            p2 = BP2.tile([128, 128], F32, tag="p2")
            nc.tensor.transpose(
                p2[:], scoresTpad[:, blk * 128:(blk + 1) * 128], if32[:])
            nc.scalar.copy(snat[:, blk * C:(blk + 1) * C], p2[:, 0:C])
        # wB = exp((snat - local_max) / sqrt(A))
        lrep = BS.tile([128, C * NB], F32, tag="lrep")
        nc.sync.dma_start(lrep[:], _ap(d_sm_in, 0, [[0, 128], [0, NB], [1, C]]))
        sd = BS.tile([128, C * NB], F32, tag="sd")
        nc.vector.tensor_tensor(sd[:], snat[:], lrep[:], op=ALU.subtract)
        wE = BS.tile([128, C * NB], F32, tag="wE")
        nc.scalar.activation(wE[:], sd[:], ACT.Exp, bias=0.0, scale=RSQA)
        nc.vector.tensor_mul(wBb[:], wE[:], padw[:])
        nc.vector.tensor_reduce(
            wsum_p[:], wBb[:].rearrange("p (b c) -> p c b", c=C),
            axis=AX.X, op=ALU.add)

    if KPH < 4:
        nc.sync.dma_start(logits_h.ap()[:], lmaxs[0:2, 0:1])
        es.close()
        return

    # ================= pass B2: bag = (w.T @ h) @ Wv' ====================
    with tc.tile_pool(name="b2_bag", bufs=1, space="PSUM") as BGP:
        pwh = BGP.tile([C, AD], F32, tag="pwh")
        pbag = [BGP.tile([C, 512], F32, tag=f"pbag{n2}", name=f"pbag{n2}") for n2 in range(2)]
        with tc.tile_pool(name="b2_ps", bufs=2, space="PSUM") as VP, \
                tc.tile_pool(name="b2_sb", bufs=4) as VS:
            for blk in range(NB):
                nc.tensor.matmul(pwh[:], wBb[:, blk * C:(blk + 1) * C],
                                 hN[:, blk * AD:(blk + 1) * AD],
                                 start=(blk == 0), stop=(blk == NB - 1))
            # wh [2, 256] -> whT [256(adapt), 2] via PE transpose
            whpad = VS.tile([128, AD], F32, tag="whpad")
            nc.vector.memset(whpad[:], 0.0)
            nc.scalar.copy(whpad[0:C, :], pwh[:])
            whT = VS.tile([128, 2 * C], F32R, tag="whT")
            for k in range(2):
                pt2 = VP.tile([128, 128], F32, tag="pt2")
                nc.tensor.transpose(pt2[:], whpad[:, k * 128:(k + 1) * 128],
                                    if32[:])
                nc.scalar.copy(whT[:, k * C:(k + 1) * C], pt2[:, 0:C])
            for n2 in range(2):
                for k in range(2):
                    nc.tensor.matmul(
                        pbag[n2][:], whT[:, k * C:(k + 1) * C],
                        wvp[:, k * D + n2 * 512: k * D + (n2 + 1) * 512],
                        start=(k == 0), stop=(k == 1))

        # ================= epilogue =================
        with tc.tile_pool(name="ep_sb", bufs=1) as S, \
                tc.tile_pool(name="ep_ps", bufs=1, space="PSUM") as EPP:
            gmaxs = S.tile([C, 1], F32, tag="gmaxs")
            nc.gpsimd.dma_start(gmaxs[:], _ap(d_sm_out, 0, [[1, C], [1, 1]]))
            gam = S.tile([C, 1], F32, tag="gam")
            nc.vector.tensor_scalar(gam[:], lmaxs[:], gmaxs[:], None,
                                    op0=ALU.subtract)
            nc.scalar.activation(gam[:], gam[:], ACT.Exp, bias=0.0, scale=RSQA)
            pws = EPP.tile([1, C], F32, tag="pws")
            nc.tensor.matmul(pws[:], ones[:], wsum_p[:],
                             start=True, stop=True)
            ws_row = S.tile([1, C], F32, tag="ws_row")
            nc.scalar.copy(ws_row[:], pws[:])
            nc.gpsimd.dma_start(_ap(d_tiny, 4, [[0, 1], [1, C]]), ws_row[:])
            ws_col = S.tile([C, 1], F32, tag="ws_col")
            nc.gpsimd.dma_start(ws_col[:], _ap(d_tiny, 4, [[1, C], [1, 1]]))
            pack = S.tile([C, D + 1], F32, tag="pack")
            for n2 in range(2):
                nc.scalar.activation(pack[:, n2 * 512:(n2 + 1) * 512],
                                     pbag[n2][:], ACT.Copy, bias=0.0,
                                     scale=gam[:])
            nc.vector.tensor_mul(pack[:, D:D + 1], ws_col[:], gam[:])
            nc.sync.dma_start(d_bag_in[:], pack[:])
            _cc(nc, ALU.add, d_bag_in, d_bag_out)
            gbag = S.tile([C, D + 1], F32, tag="gbag")
            nc.sync.dma_start(gbag[:], d_bag_out[:])
            bg = gbag[:, 0:D]
            winv = S.tile([C, 1], F32, tag="winv")
            nc.vector.reciprocal(winv[:], gbag[:, D:D + 1])
            nc.scalar.activation(bg, bg, ACT.Copy, bias=0.0, scale=winv[:])
            scr = S.tile([C, D], F32, tag="scr")
            nc.gpsimd.dma_start(_ap(d_mask, 0, [[0, 1], [1, D]]), bvp_row[:])
            nc.sync.dma_start(scr[:], _ap(d_mask, 0, [[0, C], [1, D]]))
            nc.vector.tensor_add(bg, bg, scr[:])
            # layernorm (in place on gbag[:, :D])
            mu = S.tile([C, 1], F32, tag="mu")
            nc.vector.reduce_sum(mu[:], bg, axis=AX.X)
            nc.vector.tensor_scalar_mul(mu[:], mu[:], 1.0 / D)
            nc.vector.tensor_scalar(bg, bg, mu[:], None, op0=ALU.subtract)
            var = S.tile([C, 1], F32, tag="var")
            nc.vector.tensor_mul(scr[:], bg, bg)
            nc.vector.tensor_reduce(var[:], scr[:], axis=AX.X, op=ALU.add)
            nc.vector.tensor_scalar_mul(var[:], var[:], 1.0 / D)
            sdv = S.tile([C, 1], F32, tag="sdv")
            eps = S.tile([C, 1], F32, tag="eps")
            nc.vector.memset(eps[:], 1e-5)
            nc.scalar.activation(sdv[:], var[:], ACT.Sqrt, bias=eps[:], scale=1.0)
            rinv = S.tile([C, 1], F32, tag="rinv")
            nc.vector.reciprocal(rinv[:], sdv[:])
            nc.scalar.activation(bg, bg, ACT.Copy, bias=0.0, scale=rinv[:])
            nc.vector.tensor_mul(bg, bg, g2[:])
            nc.vector.tensor_add(bg, bg, b2r[:])
            # conv contraction
            cv = S.tile([C, C], F32, tag="cv")
            for o in range(C):
                nc.vector.tensor_mul(scr[:], bg, cw[:, o * D:(o + 1) * D])
                nc.vector.tensor_reduce(cv[:, o:o + 1], scr[:],
                                        axis=AX.X, op=ALU.add)
            pcv = EPP.tile([1, C], F32, tag="pcv")
            nc.tensor.matmul(pcv[:], ones[0:C, 0:1], cv[:],
                             start=True, stop=True)
            blog_row = S.tile([1, C], F32, tag="blog_row")
            nc.scalar.copy(blog_row[:], pcv[:])
            nc.gpsimd.dma_start(_ap(d_tiny, 8, [[0, 1], [1, C]]), blog_row[:])
            blog = S.tile([C, 1], F32, tag="blog")
            nc.gpsimd.dma_start(blog[:], _ap(d_tiny, 8, [[1, C], [1, 1]]))
            nc.vector.tensor_add(blog[:], blog[:], cb[:])
            nc.vector.tensor_add(blog[:], blog[:], gmaxl[:])
            nc.scalar.activation(blog[:], blog[:], ACT.Copy, bias=0.0, scale=0.5)
            nc.gpsimd.dma_start(logits_h.ap()[:], blog[:])

    es.close()


_NC = None


def _get_nc():
    global _NC
    if _NC is None:
        _NC = build()
    return _NC


def _make_in_maps(inputs):
    x = np.asarray(inputs["x"], np.float32)
    bf = {"W1", "W2", "Wc", "Wq", "Wv"}
    w = {}
    for k in WNAMES:
        a = np.asarray(inputs[k], np.float32)
        if k == "W2":
            a = a.T          # ship W2 transposed: [D, AD]
        w[k] = np.ascontiguousarray(a.astype(ml_dtypes.bfloat16) if k in bf
                                    else a)
    in_maps = []
    xb = x.astype(ml_dtypes.bfloat16)
    for i in range(NCORES):
        xs = np.zeros((NSP, D), ml_dtypes.bfloat16)
        xs[:NS] = xb[i * NS:(i + 1) * NS]
        m = {"x": np.ascontiguousarray(xs.T)}
        m.update(w)
        in_maps.append(m)
    return in_maps


def _assemble(results):
    score = np.concatenate([results[i]["score"] for i in range(NCORES)])
    logits = np.asarray(results[0]["logits"], np.float32)
    return logits, score.astype(np.float32)


def kernel(**inputs):
    nc = _get_nc()
    res = run_bass_kernel_spmd(nc, _make_in_maps(inputs),
                               core_ids=list(range(NCORES)))
    return _assemble(res.results)


def run_traced(**inputs):
    nc = _get_nc()
    res = run_bass_kernel_spmd(nc, _make_in_maps(inputs),
                               core_ids=list(range(NCORES)), trace=True)
    return _assemble(res.results), res
